# revision 1
# baseline (speedup 1.0000x reference)
"""Trainium2 Bass kernel: 3D affine spatial transformer (affine_grid +
trilinear grid_sample, align_corners=True, zeros padding).

Data parallel: one sample per NeuronCore (8 cores). Per core, output is
processed per z-slab (96 slabs); a slab's 5120 output pixels form 128
lanes x 40 tasks of 5 consecutive x-pixels. A jy-phase-duplicated,
zero-padded fp16 copy of src in DRAM (PV[zq, y0, x0, c, jy]) lets one
indirect-DMA descriptor per (task, jz) fetch the task's full
(3z x 3y x 7x x 2c) interpolation window as a 42-element contiguous
stream. Exact trilinear weights are dense hat functions relu(1-|t|)
evaluated per task from three per-axis residuals; products reduce on
DVE in fp16 and results DMA out with an fp16->f32 cast.
"""

import numpy as np

import concourse.bass as bass
import concourse.bacc as bacc
import concourse.mybir as mybir
from concourse import tile
from concourse.bass import AP
from concourse.bass_utils import run_bass_kernel_spmd

F32 = mybir.dt.float32
F16 = mybir.dt.float16
I32 = mybir.dt.int32
AO = mybir.AluOpType
AF = mybir.ActivationFunctionType

N, C, D, H, W = 8, 2, 96, 160, 160
HW = H * W
SRCEL = D * H * W
OUTEL = C * SRCEL

LX, KX, KY, KZ = 5, 3, 3, 3
U = LX - 1 + KX                      # 7
INNER = C * KY                       # 6
DSTREAM = U * INNER                  # 42 els per (task, jz) descriptor
PADZ, PADY, PADX = 3, 4, 7
ZPN, YPN, XN = 101, 167, 176
JZSTR = YPN * XN                     # 29392 rows per z plane
ROWS = ZPN * JZSTR                   # 2968592
ROWS_PAD = 2970240
PVELS = ROWS_PAD * INNER             # 17821440 = 34 * (128*4095)
ZLIM, YLIM, XLIM = 98.0, 164.0, 169.0

TY, XIN = 20, 2
NT = TY * XIN                        # 40 tasks per lane per slab
YG, XCP = 8, 16                      # lane = yg*16 + xcp
SLABS = D
CH_SL = 12                           # slabs per phase-3 chunk
NCHUNK = SLABS // CH_SL
TPC = CH_SL * NT                     # 960 tasks/lane/chunk
NTL = SLABS * NT                     # 3840 tasks per lane

_CACHE = {}
DEBUG = False


def fb(apx, pairs, extra_off=0):
    """clone AP keeping partition pair, replacing free pairs"""
    return AP(apx.tensor, apx.offset + extra_off,
              [list(apx.ap[0])] + [list(p) for p in pairs])


def _build_program():
    P = 128
    nc = bacc.Bacc(None, target_bir_lowering=False)
    src = nc.declare_dram_parameter("src", [C, SRCEL], F32, isOutput=False)
    theta = nc.declare_dram_parameter("theta", [1, 12], F32, isOutput=False)
    out = nc.declare_dram_parameter("out", [1, OUTEL], F32, isOutput=True)
    pv = nc.dram_tensor("pv", [ROWS_PAD, INNER], F16)
    if DEBUG:
        dbg_idx = nc.declare_dram_parameter("dbg_idx", [128, NTL], F32,
                                            isOutput=True)
        dbg_r = nc.declare_dram_parameter("dbg_r", [128, NTL * 3], F16,
                                          isOutput=True)
        dbg_dt = nc.declare_dram_parameter("dbg_dt", [128, NT * KZ * DSTREAM],
                                           F16, isOutput=True)
        dbg_hat = nc.declare_dram_parameter("dbg_hat", [128, NT * 45], F16,
                                            isOutput=True)
        dbg_prod = nc.declare_dram_parameter("dbg_prod",
                                             [128, NT * C * LX * 27], F16,
                                             isOutput=True)
        dbg_red2 = nc.declare_dram_parameter("dbg_red2", [128, C * NT * LX],
                                             F16, isOutput=True)
        dbg_misc = nc.declare_dram_parameter("dbg_misc", [128, 224], F32,
                                             isOutput=True)
        dbg_pv = nc.declare_dram_parameter("dbg_pv", [128, 4096], F16,
                                           isOutput=True)

    r = 79.5 / 47.5

    with tile.TileContext(nc) as tc:
        with (
            tc.tile_pool(name="per", bufs=1) as per,
            tc.tile_pool(name="pp", bufs=1, space="PSUM") as pp,
        ):
            pre_ctx = tc.tile_pool(name="pre", bufs=2)
            pre = pre_ctx.__enter__()

            # ---------- P0: zero-fill PV ----------
            zt = pre.tile([P, 4095], F16)
            nc.vector.memset(zt[:], 0.0)
            CH = P * 4095
            for i in range(PVELS // CH):
                nc.sync.dma_start(out=AP(pv[:].tensor, i * CH, [[1, CH]]),
                                  in_=zt[:])

            # ---------- P2: scalars, templates ----------
            th0 = per.tile([P, 12], F32)
            nc.sync.dma_start(out=th0[:1, :], in_=theta[:])
            ones1 = per.tile([1, P], F32)
            nc.vector.memset(ones1[:], 1.0)
            thps = pp.tile([P, 12], F32, tag="thps")
            nc.tensor.matmul(out=thps[:], lhsT=ones1[:], rhs=th0[:1, :],
                             start=True, stop=True)
            thb = per.tile([P, 12], F32)
            nc.vector.tensor_copy(out=thb[:], in_=thps[:])

            def thc(j):
                return thb[:, j:j + 1]

            # per-axis scalars A,B,C,O' (O' includes +pad)
            sc = per.tile([P, 24], F32)
            (AZ, BZ, CZ, OZ, AY, BY, CY, OY, AX, BX, CX, OX, AXM1,
             MZB, MYB, MXB) = range(16)

            def scc(j):
                return sc[:, j:j + 1]

            tmp1 = per.tile([P, 1], F32, tag="tmp1")

            def mkrow(dst, srci, cmul, osc, oadd):
                # A,B,C,O for one axis; A=t[srci]*amul handled by caller
                nc.vector.tensor_copy(out=scc(dst[0]), in_=thc(srci))
                nc.vector.tensor_copy(out=scc(dst[1]), in_=thc(srci + 1))
                nc.vector.tensor_scalar_mul(out=scc(dst[2]),
                                            in0=thc(srci + 2), scalar1=cmul)
                nc.vector.tensor_tensor(out=tmp1[:], in0=thc(srci + 3),
                                        in1=thc(srci), op=AO.subtract)
                nc.vector.tensor_tensor(out=tmp1[:], in0=tmp1[:],
                                        in1=thc(srci + 1), op=AO.subtract)
                nc.vector.tensor_tensor(out=tmp1[:], in0=tmp1[:],
                                        in1=thc(srci + 2), op=AO.subtract)
                nc.vector.tensor_scalar(out=scc(dst[3]), in0=tmp1[:],
                                        scalar1=osc, scalar2=osc + oadd,
                                        op0=AO.mult, op1=AO.add)

            mkrow((AX, BX, CX, OX), 0, r, 79.5, float(PADX))
            mkrow((AY, BY, CY, OY), 4, r, 79.5, float(PADY))
            mkrow((AZ, BZ, CZ, OZ), 8, 1.0, 47.5, float(PADZ))
            # z row: A=t20/r, B=t21/r, C=t22
            nc.vector.tensor_scalar_mul(out=scc(AZ), in0=thc(8),
                                        scalar1=1.0 / r)
            nc.vector.tensor_scalar_mul(out=scc(BZ), in0=thc(9),
                                        scalar1=1.0 / r)
            nc.vector.tensor_scalar_add(out=scc(AXM1), in0=scc(AX),
                                        scalar1=-1.0)

            def mkminb(dst, slope_i):
                nc.vector.tensor_scalar_mul(out=scc(dst), in0=scc(slope_i),
                                            scalar1=float(LX - 1))
                nc.vector.tensor_scalar_min(out=scc(dst), in0=scc(dst),
                                            scalar1=0.0)

            mkminb(MZB, AZ)
            mkminb(MYB, AY)
            mkminb(MXB, AXM1)

            # lane mapping: flat x-chunk index g = 40*p + t;
            # y = g // 32, xc = g % 32, x0 = 5*xc
            def iotaf(shape_pairs, n, tag, base=0, cm=0):
                ti_ = pre.tile([P, n], I32, tag=tag + "i")
                nc.gpsimd.iota(ti_[:], shape_pairs, base=base,
                               channel_multiplier=cm)
                tf_ = per.tile([P, n], F32, tag=tag + "f")
                nc.vector.tensor_copy(out=tf_[:], in_=ti_[:])
                return tf_

            gT = iotaf([[1, NT]], NT, "g", cm=NT)
            yT = per.tile([P, NT], F32)
            gi1 = pre.tile([P, NT], I32, tag="gi1")
            yv = pre.tile([P, NT], F32, tag="yv")
            nc.vector.tensor_scalar_mul(out=yv[:], in0=gT[:],
                                        scalar1=1.0 / 32.0)
            nc.vector.tensor_copy(out=gi1[:], in_=yv[:])
            nc.vector.tensor_copy(out=yT[:], in_=gi1[:])
            ygt = pre.tile([P, NT], F32, tag="ygt")
            nc.vector.tensor_tensor(out=ygt[:], in0=yT[:], in1=yv[:],
                                    op=AO.is_gt)
            nc.vector.tensor_tensor(out=yT[:], in0=yT[:], in1=ygt[:],
                                    op=AO.subtract)
            x0T = per.tile([P, NT], F32)
            nc.vector.scalar_tensor_tensor(out=x0T[:], in0=yT[:],
                                           scalar=-32.0, in1=gT[:],
                                           op0=AO.mult, op1=AO.add)
            nc.vector.tensor_scalar_mul(out=x0T[:], in0=x0T[:],
                                        scalar1=float(LX))

            # base40[axis] = A*x0 + B*y + O'   [P, NT]
            base40 = per.tile([P, 3, NT], F32)
            for ax, (ai, bi, oi) in enumerate(((AZ, BZ, OZ), (AY, BY, OY),
                                               (AX, BX, OX))):
                b1 = pre.tile([P, NT], F32, tag="b1")
                nc.vector.scalar_tensor_tensor(
                    out=b1[:], in0=x0T[:], scalar=scc(ai),
                    in1=fb(sc[:], [[0, NT]], oi), op0=AO.mult, op1=AO.add)
                nc.vector.scalar_tensor_tensor(
                    out=base40[:, ax, :], in0=yT[:], scalar=scc(bi),
                    in1=b1[:], op0=AO.mult, op1=AO.add)

            # weight templates T45[axis(z,y,x), K, wl] = slope*wl - K, fp16
            iwf = iotaf([[0, 3], [0, KX], [1, LX]], 45, "iw")
            ikf = iotaf([[0, 3], [1, KX], [0, LX]], 45, "ik")
            T45f = per.tile([P, 45], F32)
            for ax, si in enumerate((AZ, AY, AXM1)):
                nc.vector.scalar_tensor_tensor(
                    out=T45f[:, ax * 15:(ax + 1) * 15],
                    in0=iwf[:, ax * 15:(ax + 1) * 15], scalar=scc(si),
                    in1=ikf[:, ax * 15:(ax + 1) * 15],
                    op0=AO.mult, op1=AO.subtract)
            T45 = per.tile([P, 45], F16)
            nc.vector.tensor_copy(out=T45[:], in_=T45f[:])

            jzi = pre.tile([P, KZ], I32, tag="jzi")
            nc.gpsimd.iota(jzi[:], [[JZSTR, KZ]], base=0, channel_multiplier=0)
            jzTf = per.tile([P, KZ], F32)
            nc.vector.tensor_copy(out=jzTf[:], in_=jzi[:])

            # ---------- P3: per-task residuals (fp16) + PV row index ----
            rT = per.tile([P, NTL, 3], F16)
            idxT = per.tile([P, NTL], F32)
            lims = (ZLIM, YLIM, XLIM)
            mbs = (MZB, MYB, MXB)
            cxs = (CZ, CY, CX)
            for ch in range(NCHUNK):
                zoi = pre.tile([P, TPC], I32, tag="zoi")
                nc.gpsimd.iota(zoi[:], [[1, CH_SL], [0, NT]],
                               base=ch * CH_SL, channel_multiplier=0)
                zof = pre.tile([P, TPC], F32, tag="zof")
                nc.vector.tensor_copy(out=zof[:], in_=zoi[:])
                acc = idxT[:, ch * TPC:(ch + 1) * TPC]
                for ax in range(3):
                    cs = pre.tile([P, TPC], F32, tag="cs")
                    nc.vector.scalar_tensor_tensor(
                        out=cs[:], in0=zof[:], scalar=scc(cxs[ax]),
                        in1=fb(base40[:, ax, :], [[0, CH_SL], [1, NT]]),
                        op0=AO.mult, op1=AO.add)
                    bf = pre.tile([P, TPC], F32, tag="bf")
                    bv = pre.tile([P, TPC], F32, tag="bv")
                    bi_ = pre.tile([P, TPC], I32, tag="bi")
                    nc.vector.tensor_scalar_add(out=bv[:], in0=cs[:],
                                                scalar1=scc(mbs[ax]))
                    nc.vector.tensor_copy(out=bi_[:], in_=bv[:])
                    nc.vector.tensor_copy(out=bf[:], in_=bi_[:])
                    bg = pre.tile([P, TPC], F32, tag="bg")
                    nc.vector.tensor_tensor(out=bg[:], in0=bf[:], in1=bv[:],
                                            op=AO.is_gt)
                    nc.vector.tensor_tensor(out=bf[:], in0=bf[:], in1=bg[:],
                                            op=AO.subtract)
                    nc.vector.tensor_scalar_max(out=bf[:], in0=bf[:],
                                                scalar1=0.0)
                    nc.vector.tensor_scalar_min(out=bf[:], in0=bf[:],
                                                scalar1=lims[ax])
                    nc.vector.tensor_tensor(
                        out=fb(rT[:], [[3, TPC]], (ch * TPC) * 3 + ax),
                        in0=cs[:], in1=bf[:], op=AO.subtract)
                    if ax == 0:
                        nc.vector.tensor_scalar_mul(out=acc, in0=bf[:],
                                                    scalar1=float(JZSTR))
                    elif ax == 1:
                        nc.vector.scalar_tensor_tensor(
                            out=acc, in0=bf[:], scalar=float(XN), in1=acc,
                            op0=AO.mult, op1=AO.add)
                    else:
                        nc.vector.tensor_tensor(out=acc, in0=acc, in1=bf[:],
                                                op=AO.add)

            if DEBUG:
                nc.sync.dma_start(
                    out=dbg_pv[:],
                    in_=AP(pv[:].tensor, (50 * YPN + 80) * XN * INNER,
                           [[1, 128 * 4096]]))
                nc.sync.dma_start(out=AP(dbg_misc[:].tensor, 0,
                                         [[2 * NT + 3 * NT + 24, 128], [1, NT]]),
                                  in_=x0T[:])
                nc.sync.dma_start(out=AP(dbg_misc[:].tensor, NT,
                                         [[2 * NT + 3 * NT + 24, 128], [1, NT]]),
                                  in_=yT[:])
                nc.sync.dma_start(out=AP(dbg_misc[:].tensor, 2 * NT,
                                         [[2 * NT + 3 * NT + 24, 128], [1, 3 * NT]]),
                                  in_=base40[:].rearrange("p a b -> p (a b)"))
                nc.sync.dma_start(out=AP(dbg_misc[:].tensor, 5 * NT,
                                         [[2 * NT + 3 * NT + 24, 128], [1, 16]]),
                                  in_=sc[:, :16])
                nc.sync.dma_start(out=dbg_idx[:], in_=idxT[:])
                nc.sync.dma_start(out=dbg_r[:],
                                  in_=rT[:].rearrange("p a b -> p (a b)"))

            # ---------- P1: build PV ----------
            # partition = z (96 rows); jy shifts live in the free dim.
            # il[z, y0p-in-band, x, c, jy] = src[c, z, y0p-4+jy, x]
            YB = 8
            bands = [(b * YB, YB) for b in range(20)] + [(160, 4)]
            for (B, nb) in bands:
                scs = []
                for c in range(C):
                    sct = pre.tile([D, (YB + 2) * W], F32, tag=f"sc{c}")
                    r0 = B - PADY                  # first src row = B-4+jy=0
                    rlo, rhi = max(0, r0), min(H, r0 + YB + 2)
                    if rlo > r0 or rhi < r0 + YB + 2:
                        nc.vector.memset(sct[:], 0.0)
                    nc.sync.dma_start(
                        out=sct[:, (rlo - r0) * W:(rhi - r0) * W],
                        in_=AP(src[:].tensor, c * SRCEL + rlo * W,
                               [[HW, D], [W, rhi - rlo], [1, W]]))
                    scs.append(sct)
                il = pre.tile([D, YB * W * INNER], F16, tag="il")
                for c in range(C):
                    for jy in range(KY):
                        nc.scalar.activation(
                            fb(il[:], [[W * INNER, nb], [INNER, W]],
                               c * KY + jy),
                            fb(scs[c][:], [[W, nb], [1, W]], jy * W),
                            AF.Copy)
                dstoff = (PADZ * YPN + B) * XN * INNER + PADX * INNER
                nc.sync.dma_start(
                    out=AP(pv[:].tensor, dstoff,
                           [[YPN * XN * INNER, D], [XN * INNER, nb],
                            [1, W * INNER]]),
                    in_=fb(il[:], [[1, nb * W * INNER]]))

            # ---------- P4: main loop over z-slabs ----------
            pre_ctx.__exit__(None, None, None)
            dctx = tc.tile_pool(name="dp", bufs=2)
            dp = dctx.__enter__()
            actx = tc.tile_pool(name="ap", bufs=2)
            apl = actx.__enter__()
            wctx = tc.tile_pool(name="wp", bufs=1)
            wp = wctx.__enter__()
            octx = tc.tile_pool(name="op", bufs=2)
            op = octx.__enter__()

            stA = {}

            def stageA(sl):
                offf = dp.tile([P, NT, KZ], F32, tag="offf")
                nc.vector.tensor_tensor(
                    out=offf[:],
                    in0=fb(idxT[:], [[1, NT], [0, KZ]], sl * NT),
                    in1=fb(jzTf[:], [[0, NT], [1, KZ]]), op=AO.add)
                offs = dp.tile([P, NT * KZ], I32, tag="offs")
                nc.vector.tensor_copy(out=offs[:], in_=offf[:].rearrange(
                    "p a b -> p (a b)"))
                Dt = dp.tile([P, NT * KZ * DSTREAM], F16, tag="Dt")
                # HW indirect DMA honors exactly one offset per partition
                # per instruction -> one instruction per (task, jz) column
                for j in range(NT * KZ):
                    nc.gpsimd.indirect_dma_start(
                        out=Dt[:, j * DSTREAM:(j + 1) * DSTREAM],
                        out_offset=None, in_=pv[:],
                        in_offset=bass.IndirectOffsetOnAxis(
                            ap=offs[:, j:j + 1], axis=0))
                args = apl.tile([P, NT, 3, 15], F16, tag="args")
                nc.vector.tensor_tensor(
                    out=args[:],
                    in0=fb(T45[:], [[0, NT], [15, 3], [1, 15]]),
                    in1=fb(rT[:], [[3, NT], [1, 3], [0, 15]], sl * NT * 3),
                    op=AO.add)
                habs = apl.tile([P, NT * 45], F16, tag="habs")
                nc.scalar.activation(habs[:],
                                     args[:].rearrange("p a b c -> p (a b c)"),
                                     AF.Abs)
                hatt = apl.tile([P, NT * 45], F16, tag="hatt")
                nc.scalar.activation(hatt[:], habs[:], AF.Relu,
                                     bias=1.0, scale=-1.0)
                hxb = apl.tile([P, NT, LX, KX, 9], F16, tag="hxb")
                for s in range(KX):
                    nc.scalar.activation(
                        fb(hxb[:], [[135, NT], [27, LX], [1, 9]], s * 9),
                        fb(hatt[:], [[45, NT], [1, LX], [0, 9]], 30 + s * 5),
                        AF.Copy)
                if DEBUG and sl == 0:
                    nc.sync.dma_start(out=dbg_dt[:], in_=Dt[:])
                    nc.sync.dma_start(out=dbg_hat[:], in_=hatt[:])
                return offf, offs, Dt, hatt, hxb

            def stageB(sl, Dt, hatt, hxb):
                # ISA limit: <=3 free dims per operand -> split small dims out
                w2 = wp.tile([P, NT, LX, KZ, KY], F16, tag="w2")
                for jz in range(KZ):
                    nc.vector.tensor_tensor(
                        out=fb(w2[:], [[45, NT], [9, LX], [1, KY]], jz * KY),
                        in0=fb(hatt[:], [[45, NT], [1, LX], [0, KY]],
                               jz * LX),
                        in1=fb(hatt[:], [[45, NT], [1, LX], [5, KY]], 15),
                        op=AO.mult)
                w3 = wp.tile([P, NT, LX, KX, 9], F16, tag="w3")
                for s in range(KX):
                    nc.vector.tensor_tensor(
                        out=fb(w3[:], [[135, NT], [27, LX], [1, 9]], s * 9),
                        in0=fb(w2[:], [[45, NT], [9, LX], [1, 9]]),
                        in1=fb(hxb[:], [[135, NT], [27, LX], [1, 9]], s * 9),
                        op=AO.mult)
                prod = wp.tile([P, NT, C, LX, 27], F16, tag="prod")
                for s in range(KX):
                    for jz in range(KZ):
                        for c in range(C):
                            nc.vector.tensor_tensor(
                                out=fb(prod[:], [[C * LX * 27, NT], [27, LX],
                                                 [1, KY]],
                                       c * LX * 27 + s * 9 + jz * 3),
                                in0=fb(w3[:], [[LX * 27, NT], [27, LX],
                                               [1, KY]], s * 9 + jz * 3),
                                in1=fb(Dt[:], [[KZ * DSTREAM, NT], [INNER, LX],
                                               [1, KY]],
                                       jz * DSTREAM + s * INNER + c * KY),
                                op=AO.mult)
                red1 = wp.tile([P, NT * C * LX * 9], F16, tag="red1")
                with nc.allow_low_precision(reason="fp16 trilinear accum"):
                    nc.vector.tensor_reduce(
                        out=red1[:],
                        in_=fb(prod[:], [[KY, NT * C * LX * 9], [1, KY]]),
                        op=AO.add, axis=mybir.AxisListType.X)
                    # red2 stored [c, t, wl] so each c is one contiguous
                    # 200-el run per lane -> clean output descriptors
                    red2 = op.tile([P, C * NT * LX], F16, tag="red2")
                    nc.vector.tensor_reduce(
                        out=fb(red2[:], [[LX, NT], [NT * LX, C], [1, LX]]),
                        in_=fb(red1[:], [[C * LX * 9, NT], [LX * 9, C],
                                         [9, LX], [1, 9]]),
                        op=AO.add, axis=mybir.AxisListType.X)
                if DEBUG and sl == 0:
                    nc.sync.dma_start(
                        out=dbg_prod[:],
                        in_=prod[:].rearrange("p a b c d -> p (a b c d)"))
                    nc.sync.dma_start(out=dbg_red2[:], in_=red2[:])
                for c in range(C):
                    nc.gpsimd.dma_start(
                        out=AP(out[:].tensor, c * SRCEL + sl * HW,
                               [[NT * LX, P], [1, NT * LX]]),
                        in_=fb(red2[:], [[1, NT * LX]], c * NT * LX))

            for sl in range(SLABS):
                a = stageA(sl)
                if sl > 0:
                    pa = stA[sl - 1]
                    stageB(sl - 1, pa[2], pa[3], pa[4])
                stA[sl] = a
            pa = stA[SLABS - 1]
            stageB(SLABS - 1, pa[2], pa[3], pa[4])

            octx.__exit__(None, None, None)
            wctx.__exit__(None, None, None)
            actx.__exit__(None, None, None)
            dctx.__exit__(None, None, None)

    nc.compile()
    return nc


def kernel(src, theta):
    if "prog" not in _CACHE:
        _CACHE["prog"] = _build_program()
    nc = _CACHE["prog"]
    in_maps = []
    for i in range(N):
        in_maps.append({
            "src": np.ascontiguousarray(src[i].reshape(C, SRCEL),
                                        dtype=np.float32),
            "theta": np.ascontiguousarray(theta[i].reshape(1, 12),
                                          dtype=np.float32),
        })
    res = run_bass_kernel_spmd(nc, in_maps, core_ids=list(range(N)))
    o = np.empty((N, C, D, H, W), dtype=np.float32)
    for i in range(N):
        o[i] = res.results[i]["out"].reshape(C, D, H, W)
    return o



# revision 8
# speedup vs baseline: 1.5024x; 1.5024x over previous
"""Trainium2 Bass kernel: 3D affine spatial transformer (affine_grid +
trilinear grid_sample, align_corners=True, zeros padding).

Data parallel: one sample per NeuronCore (8 cores). Per core, output is
processed per z-slab (96 slabs); a slab's 25600 output pixels form 128
lanes x 20 tasks of 10 consecutive x-pixels. A z- and y-duplicated,
zero-padded fp16 copy of src in DRAM (PV[zq, y, x][jz, c, jy]) lets ONE
indirect-DMA descriptor per task fetch the full (3z x 4y x 13x x 2c)
interpolation window as a 312-element contiguous stream (vs one
descriptor per (task, jz) in the previous version -> 6x fewer SWDGE
instructions, which were the bottleneck at ~1us each on the GpSimd Q7).
Blend is factored x-first: XB = sum_s hx(i,s)*D[i+s,:], then
out = sum_{jz,jy} hz*hy*XB -- exact trilinear with per-pixel hats.
"""

import numpy as np

import concourse.bass as bass
import concourse.bacc as bacc
import concourse.mybir as mybir
from concourse import tile
from concourse.bass import AP
from concourse.bass_utils import run_bass_kernel_spmd

F32 = mybir.dt.float32
F16 = mybir.dt.float16
I32 = mybir.dt.int32
AO = mybir.AluOpType
AF = mybir.ActivationFunctionType

N, C, D, H, W = 8, 2, 96, 160, 160
HW = H * W
SRCEL = D * H * W
OUTEL = C * SRCEL

LX, KX, KY, KZ = 10, 5, 4, 3
U = LX - 1 + KX                      # 14
INNER = KZ * C * KY                  # 24 per (zq,y,x) row: [jz][c][jy]
DSTREAM = U * INNER                  # 336 els per task descriptor
PADZ, PADY, PADX = 4, 6, 11
ZPN, YPN, XN = 100, 168, 185
JZSTR = YPN * XN                     # 29700 rows per z plane
ROWS = ZPN * JZSTR                   # 2940300
ZLIM, YLIM, XLIM = 99.0, 167.0, 171.0

NT = 20                              # tasks per lane per slab
TPR = W // LX                        # 16 tasks per output row
SLABS = D
CH_SL = 12                           # slabs per residual-phase chunk
NCHUNK = SLABS // CH_SL
TPC = CH_SL * NT                     # 240 tasks/lane/chunk
NTL = SLABS * NT                     # 1920 tasks per lane

HZB, HYB, HXB = 0, KZ * LX, KZ * LX + KY * LX   # 0, 30, 70
PSTR = INNER * KX                                # 120
TSTR4 = LX * PSTR                                # 1200
HATN = (KZ + KY + KX) * LX                       # 120

_CACHE = {}


def fb(apx, pairs, extra_off=0):
    """clone AP keeping partition pair, replacing free pairs"""
    return AP(apx.tensor, apx.offset + extra_off,
              [list(apx.ap[0])] + [list(p) for p in pairs])


def _build_program():
    P = 128
    nc = bacc.Bacc(None, target_bir_lowering=False)
    src = nc.declare_dram_parameter("src", [C, SRCEL], F32, isOutput=False)
    theta = nc.declare_dram_parameter("theta", [1, 12], F32, isOutput=False)
    out = nc.declare_dram_parameter("out", [1, OUTEL], F32, isOutput=True)
    src16p = nc.dram_tensor("src16p", [C, (D + PADZ + 2) * HW], F16)
    pv = nc.dram_tensor("pv", [ROWS, INNER], F16)

    r = 79.5 / 47.5

    with tile.TileContext(nc) as tc:
        with (
            tc.tile_pool(name="per", bufs=1) as per,
            tc.tile_pool(name="pp", bufs=1, space="PSUM") as pp,
        ):
            pre_ctx = tc.tile_pool(name="pre", bufs=2)
            pre = pre_ctx.__enter__()

            # ---------- P2: scalars, templates ----------
            th0 = per.tile([P, 12], F32)
            nc.sync.dma_start(out=th0[:1, :], in_=theta[:])
            ones1 = per.tile([1, P], F32)
            nc.vector.memset(ones1[:], 1.0)
            thps = pp.tile([P, 12], F32, tag="thps")
            nc.tensor.matmul(out=thps[:], lhsT=ones1[:], rhs=th0[:1, :],
                             start=True, stop=True)
            thb = per.tile([P, 12], F32)
            nc.vector.tensor_copy(out=thb[:], in_=thps[:])

            def thc(j):
                return thb[:, j:j + 1]

            # per-axis scalars A,B,C,O' (O' includes +pad)
            sc = per.tile([P, 24], F32)
            (AZ, BZ, CZ, OZ, AY, BY, CY, OY, AX, BX, CX, OX, AXM1,
             MZB, MYB, MXB) = range(16)

            def scc(j):
                return sc[:, j:j + 1]

            tmp1 = per.tile([P, 1], F32, tag="tmp1")

            def mkrow(dst, srci, cmul, osc, oadd):
                nc.vector.tensor_copy(out=scc(dst[0]), in_=thc(srci))
                nc.vector.tensor_copy(out=scc(dst[1]), in_=thc(srci + 1))
                nc.vector.tensor_scalar_mul(out=scc(dst[2]),
                                            in0=thc(srci + 2), scalar1=cmul)
                nc.vector.tensor_tensor(out=tmp1[:], in0=thc(srci + 3),
                                        in1=thc(srci), op=AO.subtract)
                nc.vector.tensor_tensor(out=tmp1[:], in0=tmp1[:],
                                        in1=thc(srci + 1), op=AO.subtract)
                nc.vector.tensor_tensor(out=tmp1[:], in0=tmp1[:],
                                        in1=thc(srci + 2), op=AO.subtract)
                nc.vector.tensor_scalar(out=scc(dst[3]), in0=tmp1[:],
                                        scalar1=osc, scalar2=osc + oadd,
                                        op0=AO.mult, op1=AO.add)

            mkrow((AX, BX, CX, OX), 0, r, 79.5, float(PADX))
            mkrow((AY, BY, CY, OY), 4, r, 79.5, float(PADY))
            mkrow((AZ, BZ, CZ, OZ), 8, 1.0, 47.5, float(PADZ))
            # z row: A=t20/r, B=t21/r, C=t22
            nc.vector.tensor_scalar_mul(out=scc(AZ), in0=thc(8),
                                        scalar1=1.0 / r)
            nc.vector.tensor_scalar_mul(out=scc(BZ), in0=thc(9),
                                        scalar1=1.0 / r)
            nc.vector.tensor_scalar_add(out=scc(AXM1), in0=scc(AX),
                                        scalar1=-1.0)

            def mkminb(dst, slope_i):
                nc.vector.tensor_scalar_mul(out=scc(dst), in0=scc(slope_i),
                                            scalar1=float(LX - 1))
                nc.vector.tensor_scalar_min(out=scc(dst), in0=scc(dst),
                                            scalar1=0.0)

            mkminb(MZB, AZ)
            mkminb(MYB, AY)
            mkminb(MXB, AXM1)

            # lane mapping: flat x-chunk index g = NT*p + t;
            # y = g // TPR, xc = g % TPR, x0 = LX*xc
            def iotaf(shape_pairs, n, tag, base=0, cm=0):
                ti_ = pre.tile([P, n], I32, tag=tag + "i")
                nc.gpsimd.iota(ti_[:], shape_pairs, base=base,
                               channel_multiplier=cm)
                tf_ = per.tile([P, n], F32, tag=tag + "f")
                nc.vector.tensor_copy(out=tf_[:], in_=ti_[:])
                return tf_

            gT = iotaf([[1, NT]], NT, "g", cm=NT)
            yT = per.tile([P, NT], F32)
            gi1 = pre.tile([P, NT], I32, tag="gi1")
            yv = pre.tile([P, NT], F32, tag="yv")
            nc.vector.tensor_scalar_mul(out=yv[:], in0=gT[:],
                                        scalar1=1.0 / TPR)
            nc.vector.tensor_copy(out=gi1[:], in_=yv[:])
            nc.vector.tensor_copy(out=yT[:], in_=gi1[:])
            ygt = pre.tile([P, NT], F32, tag="ygt")
            nc.vector.tensor_tensor(out=ygt[:], in0=yT[:], in1=yv[:],
                                    op=AO.is_gt)
            nc.vector.tensor_tensor(out=yT[:], in0=yT[:], in1=ygt[:],
                                    op=AO.subtract)
            x0T = per.tile([P, NT], F32)
            nc.vector.scalar_tensor_tensor(out=x0T[:], in0=yT[:],
                                           scalar=-float(TPR), in1=gT[:],
                                           op0=AO.mult, op1=AO.add)
            nc.vector.tensor_scalar_mul(out=x0T[:], in0=x0T[:],
                                        scalar1=float(LX))

            # base[axis] = A*x0 + B*y + O'   [P, NT]
            base3 = per.tile([P, 3, NT], F32)
            for ax, (ai, bi, oi) in enumerate(((AZ, BZ, OZ), (AY, BY, OY),
                                               (AX, BX, OX))):
                b1 = pre.tile([P, NT], F32, tag="b1")
                nc.vector.scalar_tensor_tensor(
                    out=b1[:], in0=x0T[:], scalar=scc(ai),
                    in1=fb(sc[:], [[0, NT]], oi), op0=AO.mult, op1=AO.add)
                nc.vector.scalar_tensor_tensor(
                    out=base3[:, ax, :], in0=yT[:], scalar=scc(bi),
                    in1=b1[:], op0=AO.mult, op1=AO.add)

            # hat templates T[ax][k][i] = slope_ax*i - k, fp16 [P, HATN]
            Tf = per.tile([P, HATN], F32)
            for axoff, kcnt, si in ((HZB, KZ, AZ), (HYB, KY, AY),
                                    (HXB, KX, AXM1)):
                nb_ = kcnt * LX
                iw = iotaf([[0, kcnt], [1, LX]], nb_, f"iw{axoff}")
                ik = iotaf([[1, kcnt], [0, LX]], nb_, f"ik{axoff}")
                nc.vector.scalar_tensor_tensor(
                    out=Tf[:, axoff:axoff + nb_], in0=iw[:], scalar=scc(si),
                    in1=ik[:], op0=AO.mult, op1=AO.subtract)
            T110 = per.tile([P, HATN], F16)
            nc.vector.tensor_copy(out=T110[:], in_=Tf[:])

            # ---------- P3: per-task residuals (fp16) + PV row index ----
            rT = per.tile([P, NTL, 3], F16)
            idxT = per.tile([P, NTL], F32)
            lims = (ZLIM, YLIM, XLIM)
            mbs = (MZB, MYB, MXB)
            cxs = (CZ, CY, CX)
            for ch in range(NCHUNK):
                zoi = pre.tile([P, TPC], I32, tag="zoi")
                nc.gpsimd.iota(zoi[:], [[1, CH_SL], [0, NT]],
                               base=ch * CH_SL, channel_multiplier=0)
                zof = pre.tile([P, TPC], F32, tag="zof")
                nc.vector.tensor_copy(out=zof[:], in_=zoi[:])
                acc = idxT[:, ch * TPC:(ch + 1) * TPC]
                for ax in range(3):
                    cs = pre.tile([P, TPC], F32, tag="cs")
                    nc.vector.scalar_tensor_tensor(
                        out=cs[:], in0=zof[:], scalar=scc(cxs[ax]),
                        in1=fb(base3[:, ax, :], [[0, CH_SL], [1, NT]]),
                        op0=AO.mult, op1=AO.add)
                    bf = pre.tile([P, TPC], F32, tag="bf")
                    bv = pre.tile([P, TPC], F32, tag="bv")
                    bi_ = pre.tile([P, TPC], I32, tag="bi")
                    nc.vector.tensor_scalar_add(out=bv[:], in0=cs[:],
                                                scalar1=scc(mbs[ax]))
                    nc.vector.tensor_copy(out=bi_[:], in_=bv[:])
                    nc.vector.tensor_copy(out=bf[:], in_=bi_[:])
                    bg = pre.tile([P, TPC], F32, tag="bg")
                    nc.vector.tensor_tensor(out=bg[:], in0=bf[:], in1=bv[:],
                                            op=AO.is_gt)
                    nc.vector.tensor_tensor(out=bf[:], in0=bf[:], in1=bg[:],
                                            op=AO.subtract)
                    nc.vector.tensor_scalar_max(out=bf[:], in0=bf[:],
                                                scalar1=0.0)
                    nc.vector.tensor_scalar_min(out=bf[:], in0=bf[:],
                                                scalar1=lims[ax])
                    nc.vector.tensor_tensor(
                        out=fb(rT[:], [[3, TPC]], (ch * TPC) * 3 + ax),
                        in0=cs[:], in1=bf[:], op=AO.subtract)
                    if ax == 0:
                        nc.vector.tensor_scalar_mul(out=acc, in0=bf[:],
                                                    scalar1=float(JZSTR))
                    elif ax == 1:
                        nc.vector.scalar_tensor_tensor(
                            out=acc, in0=bf[:], scalar=float(XN), in1=acc,
                            op0=AO.mult, op1=AO.add)
                    else:
                        nc.vector.tensor_tensor(out=acc, in0=acc, in1=bf[:],
                                                op=AO.add)

            # ---------- P1: cast src to fp16 (z-padded), build PV ----------
            # src16p[c] = 3 zero planes | 96 data planes | 2 zero planes
            DP5 = D + PADZ + 2
            zt = pre.tile([P, 800], F16)
            nc.vector.memset(zt[:], 0.0)
            for c in range(C):
                nc.sync.dma_start(
                    out=AP(src16p[:].tensor, c * DP5 * HW,
                           [[1, PADZ * HW]]),
                    in_=fb(zt[:], [[1, PADZ * HW // P]]))
                nc.sync.dma_start(
                    out=AP(src16p[:].tensor, c * DP5 * HW + (PADZ + D) * HW,
                           [[1, 2 * HW]]),
                    in_=fb(zt[:], [[1, 2 * HW // P]]))
            nc.gpsimd.dma_start(
                out=AP(src16p[:].tensor, PADZ * HW,
                       [[DP5 * HW, C], [1, SRCEL]]),
                in_=src[:])

            # PV row (zq, y, x)[jz][c][jy] =
            #   src16p[c, zq+jz, y-PADY+jy, x-PADX]   (padded planes)
            YB = 6
            nbands = (YPN + YB - 1) // YB
            for bidx in range(nbands):
                B = bidx * YB
                nb = min(YB, YPN - B)
                scs = []
                r0 = B - PADY                # src row at jy=0
                rlo = max(0, r0)
                rhi = min(H, r0 + nb + KY - 1)
                for c in range(C):
                    for jz in range(KZ):
                        # partition p of sct holds padded plane p + jz
                        sct = pre.tile([ZPN, (YB + KY - 1) * W], F16,
                                       tag=f"sc{c}_{jz}")
                        if rlo > r0 or rhi < r0 + nb + KY - 1:
                            nc.vector.memset(sct[:], 0.0)
                        if rhi > rlo:
                            nc.sync.dma_start(
                                out=sct[:, (rlo - r0) * W:(rhi - r0) * W],
                                in_=AP(src16p[:].tensor,
                                       c * DP5 * HW + jz * HW + rlo * W,
                                       [[HW, ZPN],
                                        [1, (rhi - rlo) * W]]))
                        scs.append(sct)
                il = pre.tile([ZPN, YB * XN * INNER], F16, tag="il")
                # zero x-pad columns (x<PADX and x>=PADX+W)
                nc.vector.memset(
                    fb(il[:], [[XN * INNER, nb], [1, PADX * INNER]]), 0.0)
                nc.vector.memset(
                    fb(il[:], [[XN * INNER, nb],
                               [1, (XN - PADX - W) * INNER]],
                       (PADX + W) * INNER), 0.0)
                # interleave: 6 copies (c, jz), jy merged via stride-W pair
                for c in range(C):
                    for jz in range(KZ):
                        dst = fb(il[:], [[XN * INNER, nb], [INNER, W],
                                         [1, KY]],
                                 PADX * INNER + jz * C * KY + c * KY)
                        srcap = fb(scs[c * KZ + jz][:],
                                   [[W, nb], [1, W], [W, KY]])
                        if (c * KZ + jz) % 2 == 0:
                            nc.scalar.activation(dst, srcap, AF.Copy)
                        else:
                            nc.vector.tensor_copy(out=dst, in_=srcap)
                nc.sync.dma_start(
                    out=AP(pv[:].tensor, B * XN * INNER,
                           [[JZSTR * INNER, ZPN], [1, nb * XN * INNER]]),
                    in_=fb(il[:], [[1, nb * XN * INNER]]))

            # ---------- P4: main loop over z-slabs ----------
            pre_ctx.__exit__(None, None, None)
            dctx = tc.tile_pool(name="dp", bufs=2)
            dp = dctx.__enter__()
            actx = tc.tile_pool(name="ap", bufs=2)
            apl = actx.__enter__()
            wctx = tc.tile_pool(name="wp", bufs=1)
            wp = wctx.__enter__()
            octx = tc.tile_pool(name="op", bufs=2)
            op = octx.__enter__()

            stA = {}

            def stageA(sl):
                offs = dp.tile([P, NT], I32, tag="offs")
                nc.vector.tensor_copy(
                    out=offs[:], in_=fb(idxT[:], [[1, NT]], sl * NT))
                Dt = dp.tile([P, NT * DSTREAM], F16, tag="Dt")
                for t in range(NT):
                    nc.gpsimd.indirect_dma_start(
                        out=Dt[:, t * DSTREAM:(t + 1) * DSTREAM],
                        out_offset=None, in_=pv[:],
                        in_offset=bass.IndirectOffsetOnAxis(
                            ap=offs[:, t:t + 1], axis=0))
                # args[t, ax-block] = T110 + r_ax ; hats = relu(1-|args|)
                args = apl.tile([P, NT, HATN], F16, tag="args")
                for axoff, kcnt, ax in ((HZB, KZ, 0), (HYB, KY, 1),
                                        (HXB, KX, 2)):
                    nb_ = kcnt * LX
                    nc.vector.tensor_tensor(
                        out=fb(args[:], [[HATN, NT], [1, nb_]], axoff),
                        in0=fb(T110[:], [[0, NT], [1, nb_]], axoff),
                        in1=fb(rT[:], [[3, NT], [0, nb_]], sl * NT * 3 + ax),
                        op=AO.add)
                habs = apl.tile([P, NT * HATN], F16, tag="habs")
                nc.scalar.activation(habs[:],
                                     args[:].rearrange("p a b -> p (a b)"),
                                     AF.Abs)
                hatt = apl.tile([P, NT * HATN], F16, tag="hatt")
                nc.scalar.activation(hatt[:], habs[:], AF.Relu,
                                     bias=1.0, scale=-1.0)
                return Dt, hatt

            def stageB(sl, Dt, hatt):
                # w2[t][i][jz][jy] = hz * hy
                w2 = wp.tile([P, NT * 120], F16, tag="w2")
                for jz in range(KZ):
                    nc.vector.tensor_tensor(
                        out=fb(w2[:], [[120, NT], [12, LX], [1, KY]], jz * 4),
                        in0=fb(hatt[:], [[HATN, NT], [1, LX], [0, KY]],
                               HZB + jz * LX),
                        in1=fb(hatt[:], [[HATN, NT], [1, LX], [LX, KY]],
                               HYB),
                        op=AO.mult)
                # prod4[t][i][(jz c jy)24][s] = D[i+s, :] * hx(i, s)
                prod4 = wp.tile([P, NT * TSTR4], F16, tag="prod4")
                for s in range(KX):
                    nc.vector.tensor_tensor(
                        out=fb(prod4[:], [[TSTR4, NT], [PSTR, LX], [KX, INNER]],
                               s),
                        in0=fb(Dt[:], [[DSTREAM, NT], [INNER, LX],
                                       [1, INNER]], s * INNER),
                        in1=fb(hatt[:], [[HATN, NT], [1, LX], [0, INNER]],
                               HXB + s * LX),
                        op=AO.mult)
                with nc.allow_low_precision(reason="fp16 trilinear accum"):
                    # XB[t][i][jz][c][jy] = sum_s prod4
                    XB = wp.tile([P, NT * 240], F16, tag="XB")
                    nc.vector.tensor_reduce(
                        out=XB[:],
                        in_=fb(prod4[:], [[TSTR4, NT], [KX, 240], [1, KX]]),
                        op=AO.add, axis=mybir.AxisListType.X)
                    # p5[t][i][c][jz][jy] = XB * w2
                    p5 = wp.tile([P, NT * 240], F16, tag="p5")
                    for jz in range(KZ):
                        for c in range(C):
                            nc.vector.tensor_tensor(
                                out=fb(p5[:], [[240, NT], [24, LX], [1, KY]],
                                       c * 12 + jz * 4),
                                in0=fb(XB[:], [[240, NT], [24, LX], [1, KY]],
                                       jz * C * KY + c * KY),
                                in1=fb(w2[:], [[120, NT], [12, LX], [1, KY]],
                                       jz * 4),
                                op=AO.mult)
                    # otile[c][t][i] = sum_{jz,jy} p5   (f32)
                    otile = op.tile([P, C * NT * LX], F32, tag="otile")
                    for c in range(C):
                        nc.vector.tensor_reduce(
                            out=fb(otile[:], [[LX, NT], [1, LX]],
                                   c * NT * LX),
                            in_=fb(p5[:], [[240, NT], [24, LX], [1, 12]],
                                   c * 12),
                            op=AO.add, axis=mybir.AxisListType.X)
                for c in range(C):
                    nc.sync.dma_start(
                        out=AP(out[:].tensor, c * SRCEL + sl * HW,
                               [[NT * LX, P], [1, NT * LX]]),
                        in_=fb(otile[:], [[1, NT * LX]], c * NT * LX))

            for sl in range(SLABS):
                a = stageA(sl)
                if sl > 0:
                    pa = stA.pop(sl - 1)
                    stageB(sl - 1, pa[0], pa[1])
                stA[sl] = a
            pa = stA.pop(SLABS - 1)
            stageB(SLABS - 1, pa[0], pa[1])

            octx.__exit__(None, None, None)
            wctx.__exit__(None, None, None)
            actx.__exit__(None, None, None)
            dctx.__exit__(None, None, None)

    nc.compile()
    return nc


def kernel(src, theta):
    if "prog" not in _CACHE:
        _CACHE["prog"] = _build_program()
    nc = _CACHE["prog"]
    in_maps = []
    for i in range(N):
        in_maps.append({
            "src": np.ascontiguousarray(src[i].reshape(C, SRCEL),
                                        dtype=np.float32),
            "theta": np.ascontiguousarray(theta[i].reshape(1, 12),
                                          dtype=np.float32),
        })
    res = run_bass_kernel_spmd(nc, in_maps, core_ids=list(range(N)))
    o = np.empty((N, C, D, H, W), dtype=np.float32)
    for i in range(N):
        o[i] = res.results[i]["out"].reshape(C, D, H, W)
    return o


# revision 9
# speedup vs baseline: 2.1027x; 1.3996x over previous
"""Trainium2 Bass kernel: 3D affine spatial transformer (affine_grid +
trilinear grid_sample, align_corners=True, zeros padding).

Data parallel: one sample per NeuronCore (8 cores). Per core, output is
processed per z-slab (96 slabs); a slab's 25600 output pixels form 128
lanes x 20 tasks of 10 consecutive x-pixels. A z- and y-duplicated,
zero-padded fp16 copy of src in DRAM (PV[zq, y, x][jz, c, jy]) lets ONE
indirect-DMA descriptor per task fetch the full (3z x 4y x 13x x 2c)
interpolation window as a 312-element contiguous stream (vs one
descriptor per (task, jz) in the previous version -> 6x fewer SWDGE
instructions, which were the bottleneck at ~1us each on the GpSimd Q7).
Blend is factored x-first: XB = sum_s hx(i,s)*D[i+s,:], then
out = sum_{jz,jy} hz*hy*XB -- exact trilinear with per-pixel hats.
"""

import numpy as np

import concourse.bass as bass
import concourse.bacc as bacc
import concourse.mybir as mybir
from concourse import tile
from concourse.bass import AP
from concourse.bass_utils import run_bass_kernel_spmd

F32 = mybir.dt.float32
F16 = mybir.dt.float16
I32 = mybir.dt.int32
AO = mybir.AluOpType
AF = mybir.ActivationFunctionType

N, C, D, H, W = 8, 2, 96, 160, 160
HW = H * W
SRCEL = D * H * W
OUTEL = C * SRCEL

LX, KX, KY, KZ = 10, 5, 4, 3
U = LX - 1 + KX                      # 14
INNER = KZ * C * KY                  # 24 per (zq,y,x) row: [jz][c][jy]
DSTREAM = U * INNER                  # 336 els per task descriptor
PADZ, PADY, PADX = 4, 6, 11
ZPN, YPN, XN = 100, 168, 185
JZSTR = YPN * XN                     # 29700 rows per z plane
ROWS = ZPN * JZSTR                   # 2940300
ZLIM, YLIM, XLIM = 99.0, 167.0, 171.0

NT = 20                              # tasks per lane per slab
TPR = W // LX                        # 16 tasks per output row
SLABS = D
CH_SL = 12                           # slabs per residual-phase chunk
NCHUNK = SLABS // CH_SL
TPC = CH_SL * NT                     # 240 tasks/lane/chunk
NTL = SLABS * NT                     # 1920 tasks per lane

HZB, HYB, HXB = 0, KZ * LX, KZ * LX + KY * LX   # 0, 30, 70
PSTR = INNER * KX                                # 120
TSTR4 = LX * PSTR                                # 1200
HATN = (KZ + KY + KX) * LX                       # 120

_CACHE = {}


def fb(apx, pairs, extra_off=0):
    """clone AP keeping partition pair, replacing free pairs"""
    return AP(apx.tensor, apx.offset + extra_off,
              [list(apx.ap[0])] + [list(p) for p in pairs])


def _build_program():
    P = 128
    nc = bacc.Bacc(None, target_bir_lowering=False)
    src = nc.declare_dram_parameter("src", [C, SRCEL], F32, isOutput=False)
    theta = nc.declare_dram_parameter("theta", [1, 12], F32, isOutput=False)
    out = nc.declare_dram_parameter("out", [1, OUTEL], F32, isOutput=True)
    src16p = nc.dram_tensor("src16p", [C, (D + PADZ + 2) * HW], F16)
    pv = nc.dram_tensor("pv", [ROWS, INNER], F16)

    r = 79.5 / 47.5

    with tile.TileContext(nc) as tc:
        with (
            tc.tile_pool(name="per", bufs=1) as per,
            tc.tile_pool(name="pp", bufs=1, space="PSUM") as pp,
        ):
            pre_ctx = tc.tile_pool(name="pre", bufs=2)
            pre = pre_ctx.__enter__()

            # ---------- P2: scalars, templates ----------
            th0 = per.tile([P, 12], F32)
            nc.sync.dma_start(out=th0[:1, :], in_=theta[:])
            ones1 = per.tile([1, P], F32)
            nc.vector.memset(ones1[:], 1.0)
            thps = pp.tile([P, 12], F32, tag="thps")
            nc.tensor.matmul(out=thps[:], lhsT=ones1[:], rhs=th0[:1, :],
                             start=True, stop=True)
            thb = per.tile([P, 12], F32)
            nc.vector.tensor_copy(out=thb[:], in_=thps[:])

            def thc(j):
                return thb[:, j:j + 1]

            # per-axis scalars A,B,C,O' (O' includes +pad)
            sc = per.tile([P, 24], F32)
            (AZ, BZ, CZ, OZ, AY, BY, CY, OY, AX, BX, CX, OX, AXM1,
             MZB, MYB, MXB) = range(16)

            def scc(j):
                return sc[:, j:j + 1]

            tmp1 = per.tile([P, 1], F32, tag="tmp1")

            def mkrow(dst, srci, cmul, osc, oadd):
                nc.vector.tensor_copy(out=scc(dst[0]), in_=thc(srci))
                nc.vector.tensor_copy(out=scc(dst[1]), in_=thc(srci + 1))
                nc.vector.tensor_scalar_mul(out=scc(dst[2]),
                                            in0=thc(srci + 2), scalar1=cmul)
                nc.vector.tensor_tensor(out=tmp1[:], in0=thc(srci + 3),
                                        in1=thc(srci), op=AO.subtract)
                nc.vector.tensor_tensor(out=tmp1[:], in0=tmp1[:],
                                        in1=thc(srci + 1), op=AO.subtract)
                nc.vector.tensor_tensor(out=tmp1[:], in0=tmp1[:],
                                        in1=thc(srci + 2), op=AO.subtract)
                nc.vector.tensor_scalar(out=scc(dst[3]), in0=tmp1[:],
                                        scalar1=osc, scalar2=osc + oadd,
                                        op0=AO.mult, op1=AO.add)

            mkrow((AX, BX, CX, OX), 0, r, 79.5, float(PADX))
            mkrow((AY, BY, CY, OY), 4, r, 79.5, float(PADY))
            mkrow((AZ, BZ, CZ, OZ), 8, 1.0, 47.5, float(PADZ))
            # z row: A=t20/r, B=t21/r, C=t22
            nc.vector.tensor_scalar_mul(out=scc(AZ), in0=thc(8),
                                        scalar1=1.0 / r)
            nc.vector.tensor_scalar_mul(out=scc(BZ), in0=thc(9),
                                        scalar1=1.0 / r)
            nc.vector.tensor_scalar_add(out=scc(AXM1), in0=scc(AX),
                                        scalar1=-1.0)

            def mkminb(dst, slope_i):
                nc.vector.tensor_scalar_mul(out=scc(dst), in0=scc(slope_i),
                                            scalar1=float(LX - 1))
                nc.vector.tensor_scalar_min(out=scc(dst), in0=scc(dst),
                                            scalar1=0.0)

            mkminb(MZB, AZ)
            mkminb(MYB, AY)
            mkminb(MXB, AXM1)

            # lane mapping: flat x-chunk index g = NT*p + t;
            # y = g // TPR, xc = g % TPR, x0 = LX*xc
            def iotaf(shape_pairs, n, tag, base=0, cm=0):
                ti_ = pre.tile([P, n], I32, tag=tag + "i")
                nc.gpsimd.iota(ti_[:], shape_pairs, base=base,
                               channel_multiplier=cm)
                tf_ = per.tile([P, n], F32, tag=tag + "f")
                nc.vector.tensor_copy(out=tf_[:], in_=ti_[:])
                return tf_

            gT = iotaf([[1, NT]], NT, "g", cm=NT)
            yT = per.tile([P, NT], F32)
            gi1 = pre.tile([P, NT], I32, tag="gi1")
            yv = pre.tile([P, NT], F32, tag="yv")
            nc.vector.tensor_scalar_mul(out=yv[:], in0=gT[:],
                                        scalar1=1.0 / TPR)
            nc.vector.tensor_copy(out=gi1[:], in_=yv[:])
            nc.vector.tensor_copy(out=yT[:], in_=gi1[:])
            ygt = pre.tile([P, NT], F32, tag="ygt")
            nc.vector.tensor_tensor(out=ygt[:], in0=yT[:], in1=yv[:],
                                    op=AO.is_gt)
            nc.vector.tensor_tensor(out=yT[:], in0=yT[:], in1=ygt[:],
                                    op=AO.subtract)
            x0T = per.tile([P, NT], F32)
            nc.vector.scalar_tensor_tensor(out=x0T[:], in0=yT[:],
                                           scalar=-float(TPR), in1=gT[:],
                                           op0=AO.mult, op1=AO.add)
            nc.vector.tensor_scalar_mul(out=x0T[:], in0=x0T[:],
                                        scalar1=float(LX))

            # base[axis] = A*x0 + B*y + O'   [P, NT]
            base3 = per.tile([P, 3, NT], F32)
            for ax, (ai, bi, oi) in enumerate(((AZ, BZ, OZ), (AY, BY, OY),
                                               (AX, BX, OX))):
                b1 = pre.tile([P, NT], F32, tag="b1")
                nc.vector.scalar_tensor_tensor(
                    out=b1[:], in0=x0T[:], scalar=scc(ai),
                    in1=fb(sc[:], [[0, NT]], oi), op0=AO.mult, op1=AO.add)
                nc.vector.scalar_tensor_tensor(
                    out=base3[:, ax, :], in0=yT[:], scalar=scc(bi),
                    in1=b1[:], op0=AO.mult, op1=AO.add)

            # hat templates T[ax][k][i] = slope_ax*i - k, fp16 [P, HATN]
            Tf = per.tile([P, HATN], F32)
            for axoff, kcnt, si in ((HZB, KZ, AZ), (HYB, KY, AY),
                                    (HXB, KX, AXM1)):
                nb_ = kcnt * LX
                iw = iotaf([[0, kcnt], [1, LX]], nb_, f"iw{axoff}")
                ik = iotaf([[1, kcnt], [0, LX]], nb_, f"ik{axoff}")
                nc.vector.scalar_tensor_tensor(
                    out=Tf[:, axoff:axoff + nb_], in0=iw[:], scalar=scc(si),
                    in1=ik[:], op0=AO.mult, op1=AO.subtract)
            T110 = per.tile([P, HATN], F16)
            nc.vector.tensor_copy(out=T110[:], in_=Tf[:])

            # ---------- P3: per-task residuals (fp16) + PV row index ----
            rT = per.tile([P, NTL, 3], F16)
            idxT = per.tile([P, NTL], F32)
            lims = (ZLIM, YLIM, XLIM)
            mbs = (MZB, MYB, MXB)
            cxs = (CZ, CY, CX)
            for ch in range(NCHUNK):
                zoi = pre.tile([P, TPC], I32, tag="zoi")
                nc.gpsimd.iota(zoi[:], [[1, CH_SL], [0, NT]],
                               base=ch * CH_SL, channel_multiplier=0)
                zof = pre.tile([P, TPC], F32, tag="zof")
                nc.vector.tensor_copy(out=zof[:], in_=zoi[:])
                acc = idxT[:, ch * TPC:(ch + 1) * TPC]
                for ax in range(3):
                    cs = pre.tile([P, TPC], F32, tag="cs")
                    nc.vector.scalar_tensor_tensor(
                        out=cs[:], in0=zof[:], scalar=scc(cxs[ax]),
                        in1=fb(base3[:, ax, :], [[0, CH_SL], [1, NT]]),
                        op0=AO.mult, op1=AO.add)
                    bf = pre.tile([P, TPC], F32, tag="bf")
                    bv = pre.tile([P, TPC], F32, tag="bv")
                    bi_ = pre.tile([P, TPC], I32, tag="bi")
                    nc.vector.tensor_scalar_add(out=bv[:], in0=cs[:],
                                                scalar1=scc(mbs[ax]))
                    nc.vector.tensor_copy(out=bi_[:], in_=bv[:])
                    nc.vector.tensor_copy(out=bf[:], in_=bi_[:])
                    bg = pre.tile([P, TPC], F32, tag="bg")
                    nc.vector.tensor_tensor(out=bg[:], in0=bf[:], in1=bv[:],
                                            op=AO.is_gt)
                    nc.vector.tensor_tensor(out=bf[:], in0=bf[:], in1=bg[:],
                                            op=AO.subtract)
                    nc.vector.tensor_scalar_max(out=bf[:], in0=bf[:],
                                                scalar1=0.0)
                    nc.vector.tensor_scalar_min(out=bf[:], in0=bf[:],
                                                scalar1=lims[ax])
                    nc.vector.tensor_tensor(
                        out=fb(rT[:], [[3, TPC]], (ch * TPC) * 3 + ax),
                        in0=cs[:], in1=bf[:], op=AO.subtract)
                    if ax == 0:
                        nc.vector.tensor_scalar_mul(out=acc, in0=bf[:],
                                                    scalar1=float(JZSTR))
                    elif ax == 1:
                        nc.vector.scalar_tensor_tensor(
                            out=acc, in0=bf[:], scalar=float(XN), in1=acc,
                            op0=AO.mult, op1=AO.add)
                    else:
                        nc.vector.tensor_tensor(out=acc, in0=acc, in1=bf[:],
                                                op=AO.add)

            # ---------- P1: cast src to fp16 (z-padded), build PV ----------
            # src16p[c] = 3 zero planes | 96 data planes | 2 zero planes
            DP5 = D + PADZ + 2
            zt = pre.tile([P, 800], F16)
            nc.vector.memset(zt[:], 0.0)
            for c in range(C):
                nc.sync.dma_start(
                    out=AP(src16p[:].tensor, c * DP5 * HW,
                           [[1, PADZ * HW]]),
                    in_=fb(zt[:], [[1, PADZ * HW // P]]))
                nc.sync.dma_start(
                    out=AP(src16p[:].tensor, c * DP5 * HW + (PADZ + D) * HW,
                           [[1, 2 * HW]]),
                    in_=fb(zt[:], [[1, 2 * HW // P]]))
            nc.gpsimd.dma_start(
                out=AP(src16p[:].tensor, PADZ * HW,
                       [[DP5 * HW, C], [1, SRCEL]]),
                in_=src[:])

            # PV row (zq, y, x)[jz][c][jy] =
            #   src16p[c, zq+jz, y-PADY+jy, x-PADX]   (padded planes)
            YB = 6
            nbands = (YPN + YB - 1) // YB
            for bidx in range(nbands):
                B = bidx * YB
                nb = min(YB, YPN - B)
                scs = []
                r0 = B - PADY                # src row at jy=0
                rlo = max(0, r0)
                rhi = min(H, r0 + nb + KY - 1)
                for c in range(C):
                    for jz in range(KZ):
                        # partition p of sct holds padded plane p + jz
                        sct = pre.tile([ZPN, (YB + KY - 1) * W], F16,
                                       tag=f"sc{c}_{jz}")
                        if rlo > r0 or rhi < r0 + nb + KY - 1:
                            nc.vector.memset(sct[:], 0.0)
                        if rhi > rlo:
                            nc.sync.dma_start(
                                out=sct[:, (rlo - r0) * W:(rhi - r0) * W],
                                in_=AP(src16p[:].tensor,
                                       c * DP5 * HW + jz * HW + rlo * W,
                                       [[HW, ZPN],
                                        [1, (rhi - rlo) * W]]))
                        scs.append(sct)
                il = pre.tile([ZPN, YB * XN * INNER], F16, tag="il")
                # zero x-pad columns (x<PADX and x>=PADX+W)
                nc.vector.memset(
                    fb(il[:], [[XN * INNER, nb], [1, PADX * INNER]]), 0.0)
                nc.vector.memset(
                    fb(il[:], [[XN * INNER, nb],
                               [1, (XN - PADX - W) * INNER]],
                       (PADX + W) * INNER), 0.0)
                # interleave: 6 copies (c, jz), jy merged via stride-W pair
                for c in range(C):
                    for jz in range(KZ):
                        dst = fb(il[:], [[XN * INNER, nb], [INNER, W],
                                         [1, KY]],
                                 PADX * INNER + jz * C * KY + c * KY)
                        srcap = fb(scs[c * KZ + jz][:],
                                   [[W, nb], [1, W], [W, KY]])
                        if (c * KZ + jz) % 2 == 0:
                            nc.scalar.activation(dst, srcap, AF.Copy)
                        else:
                            nc.vector.tensor_copy(out=dst, in_=srcap)
                nc.sync.dma_start(
                    out=AP(pv[:].tensor, B * XN * INNER,
                           [[JZSTR * INNER, ZPN], [1, nb * XN * INNER]]),
                    in_=fb(il[:], [[1, nb * XN * INNER]]))

            # ---------- P4: main loop over z-slabs ----------
            pre_ctx.__exit__(None, None, None)
            dctx = tc.tile_pool(name="dp", bufs=2)
            dp = dctx.__enter__()
            actx = tc.tile_pool(name="ap", bufs=2)
            apl = actx.__enter__()
            wctx = tc.tile_pool(name="wp", bufs=1)
            wp = wctx.__enter__()
            octx = tc.tile_pool(name="op", bufs=2)
            op = octx.__enter__()

            stA = {}

            def stageA(sl):
                offs = dp.tile([P, NT], I32, tag="offs")
                nc.vector.tensor_copy(
                    out=offs[:], in_=fb(idxT[:], [[1, NT]], sl * NT))
                Dt = dp.tile([P, NT * DSTREAM], F16, tag="Dt")
                for t in range(NT):
                    nc.gpsimd.indirect_dma_start(
                        out=Dt[:, t * DSTREAM:(t + 1) * DSTREAM],
                        out_offset=None, in_=pv[:],
                        in_offset=bass.IndirectOffsetOnAxis(
                            ap=offs[:, t:t + 1], axis=0))
                # args[t, ax-block] = T110 + r_ax ; hats = relu(1-|args|)
                args = apl.tile([P, NT, HATN], F16, tag="args")
                for axoff, kcnt, ax in ((HZB, KZ, 0), (HYB, KY, 1),
                                        (HXB, KX, 2)):
                    nb_ = kcnt * LX
                    nc.vector.tensor_tensor(
                        out=fb(args[:], [[HATN, NT], [1, nb_]], axoff),
                        in0=fb(T110[:], [[0, NT], [1, nb_]], axoff),
                        in1=fb(rT[:], [[3, NT], [0, nb_]], sl * NT * 3 + ax),
                        op=AO.add)
                habs = apl.tile([P, NT * HATN], F16, tag="habs")
                nc.scalar.activation(habs[:],
                                     args[:].rearrange("p a b -> p (a b)"),
                                     AF.Abs)
                hatt = apl.tile([P, NT * HATN], F16, tag="hatt")
                nc.scalar.activation(hatt[:], habs[:], AF.Relu,
                                     bias=1.0, scale=-1.0)
                return Dt, hatt

            def stageB(sl, Dt, hatt):
                # w2cd[t][i][jz][c][jy] = hz * hy   (c-duplicated)
                w2cd = wp.tile([P, NT * 240], F16, tag="w2cd")
                for jz in range(KZ):
                    for c in range(C):
                        nc.vector.tensor_tensor(
                            out=fb(w2cd[:], [[240, NT], [24, LX], [1, KY]],
                                   jz * C * KY + c * KY),
                            in0=fb(hatt[:], [[HATN, NT], [1, LX], [0, KY]],
                                   HZB + jz * LX),
                            in1=fb(hatt[:], [[HATN, NT], [1, LX], [LX, KY]],
                                   HYB),
                            op=AO.mult)
                # prod4[t][s][i][(jz c jy)24] = D[i+s, :] * hx(i, s)
                # contiguous 240-el s-slices (step-1 inner keeps DVE in 2x)
                prod4 = wp.tile([P, NT * TSTR4], F16, tag="prod4")
                for s in range(KX):
                    nc.vector.tensor_tensor(
                        out=fb(prod4[:], [[TSTR4, NT], [INNER, LX],
                                          [1, INNER]], s * LX * INNER),
                        in0=fb(Dt[:], [[DSTREAM, NT], [INNER, LX],
                                       [1, INNER]], s * INNER),
                        in1=fb(hatt[:], [[HATN, NT], [1, LX], [0, INNER]],
                               HXB + s * LX),
                        op=AO.mult)
                with nc.allow_low_precision(reason="fp16 trilinear accum"):
                    # XB[t][i][jz][c][jy] = sum_s prod4 (contiguous adds)
                    XB = wp.tile([P, NT * 240], F16, tag="XB")
                    nc.vector.tensor_tensor(
                        out=XB[:],
                        in0=fb(prod4[:], [[TSTR4, NT], [1, 240]]),
                        in1=fb(prod4[:], [[TSTR4, NT], [1, 240]], 240),
                        op=AO.add)
                    for s in range(2, KX):
                        nc.vector.tensor_tensor(
                            out=XB[:], in0=XB[:],
                            in1=fb(prod4[:], [[TSTR4, NT], [1, 240]],
                                   s * 240),
                            op=AO.add)
                    # p5 = XB * w2cd  (fully contiguous)
                    p5 = wp.tile([P, NT * 240], F16, tag="p5")
                    nc.vector.tensor_tensor(out=p5[:], in0=XB[:],
                                            in1=w2cd[:], op=AO.mult)
                    # R1[t][(i jz c)60] = sum_jy p5
                    R1 = wp.tile([P, NT * 60], F16, tag="R1")
                    nc.vector.tensor_reduce(
                        out=R1[:],
                        in_=fb(p5[:], [[240, NT], [4, 60], [1, KY]]),
                        op=AO.add, axis=mybir.AxisListType.X)
                    # otile[c][t][i] = sum_jz R1   (f32)
                    otile = op.tile([P, C * NT * LX], F32, tag="otile")
                    for c in range(C):
                        nc.vector.tensor_reduce(
                            out=fb(otile[:], [[LX, NT], [1, LX]],
                                   c * NT * LX),
                            in_=fb(R1[:], [[60, NT], [6, LX], [2, KZ]], c),
                            op=AO.add, axis=mybir.AxisListType.X)
                for c in range(C):
                    nc.sync.dma_start(
                        out=AP(out[:].tensor, c * SRCEL + sl * HW,
                               [[NT * LX, P], [1, NT * LX]]),
                        in_=fb(otile[:], [[1, NT * LX]], c * NT * LX))

            for sl in range(SLABS):
                a = stageA(sl)
                if sl > 0:
                    pa = stA.pop(sl - 1)
                    stageB(sl - 1, pa[0], pa[1])
                stA[sl] = a
            pa = stA.pop(SLABS - 1)
            stageB(SLABS - 1, pa[0], pa[1])

            octx.__exit__(None, None, None)
            wctx.__exit__(None, None, None)
            actx.__exit__(None, None, None)
            dctx.__exit__(None, None, None)

    nc.compile()
    return nc


def kernel(src, theta):
    if "prog" not in _CACHE:
        _CACHE["prog"] = _build_program()
    nc = _CACHE["prog"]
    in_maps = []
    for i in range(N):
        in_maps.append({
            "src": np.ascontiguousarray(src[i].reshape(C, SRCEL),
                                        dtype=np.float32),
            "theta": np.ascontiguousarray(theta[i].reshape(1, 12),
                                          dtype=np.float32),
        })
    res = run_bass_kernel_spmd(nc, in_maps, core_ids=list(range(N)))
    o = np.empty((N, C, D, H, W), dtype=np.float32)
    for i in range(N):
        o[i] = res.results[i]["out"].reshape(C, D, H, W)
    return o


# revision 11
# speedup vs baseline: 2.4207x; 1.1512x over previous
"""Trainium2 Bass kernel: 3D affine spatial transformer (affine_grid +
trilinear grid_sample, align_corners=True, zeros padding).

Data parallel: one sample per NeuronCore (8 cores). Per core, output is
processed per z-slab (96 slabs); a slab's 25600 output pixels form 128
lanes x 20 tasks of 10 consecutive x-pixels. A z- and y-duplicated,
zero-padded fp16 copy of src in DRAM (PV[zq, y, x][jz, c, jy]) lets ONE
indirect-DMA descriptor per task fetch the full (3z x 4y x 13x x 2c)
interpolation window as a 312-element contiguous stream (vs one
descriptor per (task, jz) in the previous version -> 6x fewer SWDGE
instructions, which were the bottleneck at ~1us each on the GpSimd Q7).
Blend is factored x-first: XB = sum_s hx(i,s)*D[i+s,:], then
out = sum_{jz,jy} hz*hy*XB -- exact trilinear with per-pixel hats.
"""

import numpy as np

import concourse.bass as bass
import concourse.bacc as bacc
import concourse.mybir as mybir
from concourse import tile
from concourse.bass import AP
from concourse.bass_utils import run_bass_kernel_spmd

F32 = mybir.dt.float32
F16 = mybir.dt.float16
I32 = mybir.dt.int32
AO = mybir.AluOpType
AF = mybir.ActivationFunctionType

N, C, D, H, W = 8, 2, 96, 160, 160
HW = H * W
SRCEL = D * H * W
OUTEL = C * SRCEL

LX, KX, KY, KZ = 10, 5, 4, 3
U = LX - 1 + KX                      # 14
INNER = KZ * C * KY                  # 24 per (zq,y,x) row: [jz][c][jy]
DSTREAM = U * INNER                  # 336 els per task descriptor
PADZ, PADY, PADX = 4, 6, 11
ZPN, YPN, XN = 100, 168, 185
JZSTR = YPN * XN                     # 29700 rows per z plane
ROWS = ZPN * JZSTR                   # 2940300
ZLIM, YLIM, XLIM = 99.0, 167.0, 171.0

NT = 20                              # tasks per lane per slab
TPR = W // LX                        # 16 tasks per output row
SLABS = D
CH_SL = 12                           # slabs per residual-phase chunk
NCHUNK = SLABS // CH_SL
TPC = CH_SL * NT                     # 240 tasks/lane/chunk
NTL = SLABS * NT                     # 1920 tasks per lane

HZB, HYB, HXB = 0, KZ * LX, KZ * LX + KY * LX   # 0, 30, 70
PSTR = INNER * KX                                # 120
TSTR4 = LX * PSTR                                # 1200
HATN = (KZ + KY + KX) * LX                       # 120

_CACHE = {}


def fb(apx, pairs, extra_off=0):
    """clone AP keeping partition pair, replacing free pairs"""
    return AP(apx.tensor, apx.offset + extra_off,
              [list(apx.ap[0])] + [list(p) for p in pairs])


def _build_program():
    P = 128
    nc = bacc.Bacc(None, target_bir_lowering=False)
    src = nc.declare_dram_parameter("src", [C, SRCEL], F32, isOutput=False)
    theta = nc.declare_dram_parameter("theta", [1, 12], F32, isOutput=False)
    out = nc.declare_dram_parameter("out", [1, OUTEL], F32, isOutput=True)
    src16p = nc.dram_tensor("src16p", [C, (D + PADZ + 2) * HW], F16)
    pv = nc.dram_tensor("pv", [ROWS, INNER], F16)

    r = 79.5 / 47.5

    with tile.TileContext(nc) as tc:
        with (
            tc.tile_pool(name="per", bufs=1) as per,
            tc.tile_pool(name="pp", bufs=1, space="PSUM") as pp,
        ):
            pre_ctx = tc.tile_pool(name="pre", bufs=2)
            pre = pre_ctx.__enter__()

            # ---------- P2: scalars, templates ----------
            th0 = per.tile([P, 12], F32)
            nc.sync.dma_start(out=th0[:1, :], in_=theta[:])
            ones1 = per.tile([1, P], F32)
            nc.vector.memset(ones1[:], 1.0)
            thps = pp.tile([P, 12], F32, tag="thps")
            nc.tensor.matmul(out=thps[:], lhsT=ones1[:], rhs=th0[:1, :],
                             start=True, stop=True)
            thb = per.tile([P, 12], F32)
            nc.vector.tensor_copy(out=thb[:], in_=thps[:])

            def thc(j):
                return thb[:, j:j + 1]

            # per-axis scalars A,B,C,O' (O' includes +pad)
            sc = per.tile([P, 24], F32)
            (AZ, BZ, CZ, OZ, AY, BY, CY, OY, AX, BX, CX, OX, AXM1,
             MZB, MYB, MXB) = range(16)

            def scc(j):
                return sc[:, j:j + 1]

            tmp1 = per.tile([P, 1], F32, tag="tmp1")

            def mkrow(dst, srci, cmul, osc, oadd):
                nc.vector.tensor_copy(out=scc(dst[0]), in_=thc(srci))
                nc.vector.tensor_copy(out=scc(dst[1]), in_=thc(srci + 1))
                nc.vector.tensor_scalar_mul(out=scc(dst[2]),
                                            in0=thc(srci + 2), scalar1=cmul)
                nc.vector.tensor_tensor(out=tmp1[:], in0=thc(srci + 3),
                                        in1=thc(srci), op=AO.subtract)
                nc.vector.tensor_tensor(out=tmp1[:], in0=tmp1[:],
                                        in1=thc(srci + 1), op=AO.subtract)
                nc.vector.tensor_tensor(out=tmp1[:], in0=tmp1[:],
                                        in1=thc(srci + 2), op=AO.subtract)
                nc.vector.tensor_scalar(out=scc(dst[3]), in0=tmp1[:],
                                        scalar1=osc, scalar2=osc + oadd,
                                        op0=AO.mult, op1=AO.add)

            mkrow((AX, BX, CX, OX), 0, r, 79.5, float(PADX))
            mkrow((AY, BY, CY, OY), 4, r, 79.5, float(PADY))
            mkrow((AZ, BZ, CZ, OZ), 8, 1.0, 47.5, float(PADZ))
            # z row: A=t20/r, B=t21/r, C=t22
            nc.vector.tensor_scalar_mul(out=scc(AZ), in0=thc(8),
                                        scalar1=1.0 / r)
            nc.vector.tensor_scalar_mul(out=scc(BZ), in0=thc(9),
                                        scalar1=1.0 / r)
            nc.vector.tensor_scalar_add(out=scc(AXM1), in0=scc(AX),
                                        scalar1=-1.0)

            def mkminb(dst, slope_i):
                nc.vector.tensor_scalar_mul(out=scc(dst), in0=scc(slope_i),
                                            scalar1=float(LX - 1))
                nc.vector.tensor_scalar_min(out=scc(dst), in0=scc(dst),
                                            scalar1=0.0)

            mkminb(MZB, AZ)
            mkminb(MYB, AY)
            mkminb(MXB, AXM1)

            # lane mapping: flat x-chunk index g = NT*p + t;
            # y = g // TPR, xc = g % TPR, x0 = LX*xc
            def iotaf(shape_pairs, n, tag, base=0, cm=0):
                ti_ = pre.tile([P, n], I32, tag=tag + "i")
                nc.gpsimd.iota(ti_[:], shape_pairs, base=base,
                               channel_multiplier=cm)
                tf_ = per.tile([P, n], F32, tag=tag + "f")
                nc.vector.tensor_copy(out=tf_[:], in_=ti_[:])
                return tf_

            gT = iotaf([[1, NT]], NT, "g", cm=NT)
            yT = per.tile([P, NT], F32)
            gi1 = pre.tile([P, NT], I32, tag="gi1")
            yv = pre.tile([P, NT], F32, tag="yv")
            nc.vector.tensor_scalar_mul(out=yv[:], in0=gT[:],
                                        scalar1=1.0 / TPR)
            nc.vector.tensor_copy(out=gi1[:], in_=yv[:])
            nc.vector.tensor_copy(out=yT[:], in_=gi1[:])
            ygt = pre.tile([P, NT], F32, tag="ygt")
            nc.vector.tensor_tensor(out=ygt[:], in0=yT[:], in1=yv[:],
                                    op=AO.is_gt)
            nc.vector.tensor_tensor(out=yT[:], in0=yT[:], in1=ygt[:],
                                    op=AO.subtract)
            x0T = per.tile([P, NT], F32)
            nc.vector.scalar_tensor_tensor(out=x0T[:], in0=yT[:],
                                           scalar=-float(TPR), in1=gT[:],
                                           op0=AO.mult, op1=AO.add)
            nc.vector.tensor_scalar_mul(out=x0T[:], in0=x0T[:],
                                        scalar1=float(LX))

            # base[axis] = A*x0 + B*y + O'   [P, NT]
            base3 = per.tile([P, 3, NT], F32)
            for ax, (ai, bi, oi) in enumerate(((AZ, BZ, OZ), (AY, BY, OY),
                                               (AX, BX, OX))):
                b1 = pre.tile([P, NT], F32, tag="b1")
                nc.vector.scalar_tensor_tensor(
                    out=b1[:], in0=x0T[:], scalar=scc(ai),
                    in1=fb(sc[:], [[0, NT]], oi), op0=AO.mult, op1=AO.add)
                nc.vector.scalar_tensor_tensor(
                    out=base3[:, ax, :], in0=yT[:], scalar=scc(bi),
                    in1=b1[:], op0=AO.mult, op1=AO.add)

            # hat templates T[ax][k][i] = slope_ax*i - k, fp16 [P, HATN]
            Tf = per.tile([P, HATN], F32)
            for axoff, kcnt, si in ((HZB, KZ, AZ), (HYB, KY, AY),
                                    (HXB, KX, AXM1)):
                nb_ = kcnt * LX
                iw = iotaf([[0, kcnt], [1, LX]], nb_, f"iw{axoff}")
                ik = iotaf([[1, kcnt], [0, LX]], nb_, f"ik{axoff}")
                nc.vector.scalar_tensor_tensor(
                    out=Tf[:, axoff:axoff + nb_], in0=iw[:], scalar=scc(si),
                    in1=ik[:], op0=AO.mult, op1=AO.subtract)
            T110 = per.tile([P, HATN], F16)
            nc.vector.tensor_copy(out=T110[:], in_=Tf[:])

            # ---------- P3: per-task residuals (fp16) + PV row index ----
            rT = per.tile([P, NTL, 3], F16)
            idxT = per.tile([P, NTL], F32)
            lims = (ZLIM, YLIM, XLIM)
            mbs = (MZB, MYB, MXB)
            cxs = (CZ, CY, CX)
            for ch in range(NCHUNK):
                zoi = pre.tile([P, TPC], I32, tag="zoi")
                nc.gpsimd.iota(zoi[:], [[1, CH_SL], [0, NT]],
                               base=ch * CH_SL, channel_multiplier=0)
                zof = pre.tile([P, TPC], F32, tag="zof")
                nc.vector.tensor_copy(out=zof[:], in_=zoi[:])
                acc = idxT[:, ch * TPC:(ch + 1) * TPC]
                for ax in range(3):
                    cs = pre.tile([P, TPC], F32, tag="cs")
                    nc.vector.scalar_tensor_tensor(
                        out=cs[:], in0=zof[:], scalar=scc(cxs[ax]),
                        in1=fb(base3[:, ax, :], [[0, CH_SL], [1, NT]]),
                        op0=AO.mult, op1=AO.add)
                    bf = pre.tile([P, TPC], F32, tag="bf")
                    bv = pre.tile([P, TPC], F32, tag="bv")
                    bi_ = pre.tile([P, TPC], I32, tag="bi")
                    nc.vector.tensor_scalar_add(out=bv[:], in0=cs[:],
                                                scalar1=scc(mbs[ax]))
                    nc.vector.tensor_copy(out=bi_[:], in_=bv[:])
                    nc.vector.tensor_copy(out=bf[:], in_=bi_[:])
                    bg = pre.tile([P, TPC], F32, tag="bg")
                    nc.vector.tensor_tensor(out=bg[:], in0=bf[:], in1=bv[:],
                                            op=AO.is_gt)
                    nc.vector.tensor_tensor(out=bf[:], in0=bf[:], in1=bg[:],
                                            op=AO.subtract)
                    nc.vector.tensor_scalar_max(out=bf[:], in0=bf[:],
                                                scalar1=0.0)
                    nc.vector.tensor_scalar_min(out=bf[:], in0=bf[:],
                                                scalar1=lims[ax])
                    nc.vector.tensor_tensor(
                        out=fb(rT[:], [[3, TPC]], (ch * TPC) * 3 + ax),
                        in0=cs[:], in1=bf[:], op=AO.subtract)
                    if ax == 0:
                        nc.vector.tensor_scalar_mul(out=acc, in0=bf[:],
                                                    scalar1=float(JZSTR))
                    elif ax == 1:
                        nc.vector.scalar_tensor_tensor(
                            out=acc, in0=bf[:], scalar=float(XN), in1=acc,
                            op0=AO.mult, op1=AO.add)
                    else:
                        nc.vector.tensor_tensor(out=acc, in0=acc, in1=bf[:],
                                                op=AO.add)

            # ---------- P1: cast src to fp16 (z-padded), build PV ----------
            # src16p[c] = 3 zero planes | 96 data planes | 2 zero planes
            DP5 = D + PADZ + 2
            zt = pre.tile([P, 800], F16)
            nc.vector.memset(zt[:], 0.0)
            for c in range(C):
                nc.sync.dma_start(
                    out=AP(src16p[:].tensor, c * DP5 * HW,
                           [[1, PADZ * HW]]),
                    in_=fb(zt[:], [[1, PADZ * HW // P]]))
                nc.sync.dma_start(
                    out=AP(src16p[:].tensor, c * DP5 * HW + (PADZ + D) * HW,
                           [[1, 2 * HW]]),
                    in_=fb(zt[:], [[1, 2 * HW // P]]))
            nc.gpsimd.dma_start(
                out=AP(src16p[:].tensor, PADZ * HW,
                       [[DP5 * HW, C], [1, SRCEL]]),
                in_=src[:])

            # PV row (zq, y, x)[jz][c][jy] =
            #   src16p[c, zq+jz, y-PADY+jy, x-PADX]   (padded planes)
            YB = 6
            nbands = (YPN + YB - 1) // YB
            for bidx in range(nbands):
                B = bidx * YB
                nb = min(YB, YPN - B)
                scs = []
                r0 = B - PADY                # src row at jy=0
                rlo = max(0, r0)
                rhi = min(H, r0 + nb + KY - 1)
                for c in range(C):
                    for jz in range(KZ):
                        # partition p of sct holds padded plane p + jz
                        sct = pre.tile([ZPN, (YB + KY - 1) * W], F16,
                                       tag=f"sc{c}_{jz}")
                        if rlo > r0 or rhi < r0 + nb + KY - 1:
                            nc.vector.memset(sct[:], 0.0)
                        if rhi > rlo:
                            nc.sync.dma_start(
                                out=sct[:, (rlo - r0) * W:(rhi - r0) * W],
                                in_=AP(src16p[:].tensor,
                                       c * DP5 * HW + jz * HW + rlo * W,
                                       [[HW, ZPN],
                                        [1, (rhi - rlo) * W]]))
                        scs.append(sct)
                il = pre.tile([ZPN, YB * XN * INNER], F16, tag="il")
                # zero x-pad columns (x<PADX and x>=PADX+W)
                nc.vector.memset(
                    fb(il[:], [[XN * INNER, nb], [1, PADX * INNER]]), 0.0)
                nc.vector.memset(
                    fb(il[:], [[XN * INNER, nb],
                               [1, (XN - PADX - W) * INNER]],
                       (PADX + W) * INNER), 0.0)
                # interleave: 6 copies (c, jz), jy merged via stride-W pair
                for c in range(C):
                    for jz in range(KZ):
                        dst = fb(il[:], [[XN * INNER, nb], [INNER, W],
                                         [1, KY]],
                                 PADX * INNER + jz * C * KY + c * KY)
                        srcap = fb(scs[c * KZ + jz][:],
                                   [[W, nb], [1, W], [W, KY]])
                        if (c * KZ + jz) % 2 == 0:
                            nc.scalar.activation(dst, srcap, AF.Copy)
                        else:
                            nc.vector.tensor_copy(out=dst, in_=srcap)
                nc.sync.dma_start(
                    out=AP(pv[:].tensor, B * XN * INNER,
                           [[JZSTR * INNER, ZPN], [1, nb * XN * INNER]]),
                    in_=fb(il[:], [[1, nb * XN * INNER]]))

            # ---------- P4: main loop over z-slabs ----------
            pre_ctx.__exit__(None, None, None)
            dctx = tc.tile_pool(name="dp", bufs=3)
            dp = dctx.__enter__()
            actx = tc.tile_pool(name="ap", bufs=2)
            apl = actx.__enter__()
            wctx = tc.tile_pool(name="wp", bufs=1)
            wp = wctx.__enter__()
            octx = tc.tile_pool(name="op", bufs=2)
            op = octx.__enter__()

            stA = {}

            def stageA(sl):
                offs = dp.tile([P, NT], I32, tag="offs")
                nc.vector.tensor_copy(
                    out=offs[:], in_=fb(idxT[:], [[1, NT]], sl * NT))
                Dt = dp.tile([P, NT * DSTREAM], F16, tag="Dt")
                for t in range(NT):
                    nc.gpsimd.indirect_dma_start(
                        out=Dt[:, t * DSTREAM:(t + 1) * DSTREAM],
                        out_offset=None, in_=pv[:],
                        in_offset=bass.IndirectOffsetOnAxis(
                            ap=offs[:, t:t + 1], axis=0))
                # args[t, ax-block] = T110 + r_ax ; hats = relu(1-|args|)
                args = apl.tile([P, NT, HATN], F16, tag="args")
                for axoff, kcnt, ax in ((HZB, KZ, 0), (HYB, KY, 1),
                                        (HXB, KX, 2)):
                    nb_ = kcnt * LX
                    nc.vector.tensor_tensor(
                        out=fb(args[:], [[HATN, NT], [1, nb_]], axoff),
                        in0=fb(T110[:], [[0, NT], [1, nb_]], axoff),
                        in1=fb(rT[:], [[3, NT], [0, nb_]], sl * NT * 3 + ax),
                        op=AO.add)
                habs = apl.tile([P, NT * HATN], F16, tag="habs")
                nc.scalar.activation(habs[:],
                                     args[:].rearrange("p a b -> p (a b)"),
                                     AF.Abs)
                hatt = apl.tile([P, NT * HATN], F16, tag="hatt")
                nc.scalar.activation(hatt[:], habs[:], AF.Relu,
                                     bias=1.0, scale=-1.0)
                return Dt, hatt

            def stageB(sl, Dt, hatt):
                # w2cd[t][i][jz][c][jy] = hz * hy   (c-duplicated)
                w2cd = wp.tile([P, NT * 240], F16, tag="w2cd")
                for jz in range(KZ):
                    for c in range(C):
                        nc.vector.tensor_tensor(
                            out=fb(w2cd[:], [[240, NT], [24, LX], [1, KY]],
                                   jz * C * KY + c * KY),
                            in0=fb(hatt[:], [[HATN, NT], [1, LX], [0, KY]],
                                   HZB + jz * LX),
                            in1=fb(hatt[:], [[HATN, NT], [1, LX], [LX, KY]],
                                   HYB),
                            op=AO.mult)
                # G[t][s][i][(jz c jy)24] = D[i+s, :] * (hz*hy)   (all
                # stride-1 operands keep DVE in 2x mode)
                G = wp.tile([P, NT * TSTR4], F16, tag="G")
                for s in range(KX):
                    nc.vector.tensor_tensor(
                        out=fb(G[:], [[TSTR4, NT], [INNER, LX],
                                      [1, INNER]], s * LX * INNER),
                        in0=fb(Dt[:], [[DSTREAM, NT], [INNER, LX],
                                       [1, INNER]], s * INNER),
                        in1=fb(w2cd[:], [[240, NT], [24, LX], [1, INNER]]),
                        op=AO.mult)
                with nc.allow_low_precision(reason="fp16 trilinear accum"):
                    # R1[t][(s i jz c)300] = sum_jy G
                    R1 = wp.tile([P, NT * KX * 60], F16, tag="R1")
                    nc.vector.tensor_reduce(
                        out=R1[:],
                        in_=fb(G[:], [[TSTR4, NT], [4, KX * 60], [1, KY]]),
                        op=AO.add, axis=mybir.AxisListType.X)
                    # X3[t][(i jz c)60] = sum_s hx(i,s) * R1[s]
                    Hp = wp.tile([P, NT * KX * 60], F16, tag="Hp")
                    for s in range(KX):
                        nc.vector.tensor_tensor(
                            out=fb(Hp[:], [[KX * 60, NT], [6, LX], [1, 6]],
                                   s * 60),
                            in0=fb(R1[:], [[KX * 60, NT], [6, LX], [1, 6]],
                                   s * 60),
                            in1=fb(hatt[:], [[HATN, NT], [1, LX], [0, 6]],
                                   HXB + s * LX),
                            op=AO.mult)
                    X3 = wp.tile([P, NT * 60], F16, tag="X3")
                    nc.vector.tensor_tensor(
                        out=X3[:],
                        in0=fb(Hp[:], [[KX * 60, NT], [1, 60]]),
                        in1=fb(Hp[:], [[KX * 60, NT], [1, 60]], 60),
                        op=AO.add)
                    for s in range(2, KX):
                        nc.vector.tensor_tensor(
                            out=X3[:], in0=X3[:],
                            in1=fb(Hp[:], [[KX * 60, NT], [1, 60]], s * 60),
                            op=AO.add)
                    # otile[c][t][i] = sum_jz X3   (f32)
                    otile = op.tile([P, C * NT * LX], F32, tag="otile")
                    for c in range(C):
                        nc.vector.tensor_reduce(
                            out=fb(otile[:], [[LX, NT], [1, LX]],
                                   c * NT * LX),
                            in_=fb(X3[:], [[60, NT], [6, LX], [2, KZ]], c),
                            op=AO.add, axis=mybir.AxisListType.X)
                for c in range(C):
                    nc.sync.dma_start(
                        out=AP(out[:].tensor, c * SRCEL + sl * HW,
                               [[NT * LX, P], [1, NT * LX]]),
                        in_=fb(otile[:], [[1, NT * LX]], c * NT * LX))

            for sl in range(SLABS):
                a = stageA(sl)
                if sl > 0:
                    pa = stA.pop(sl - 1)
                    stageB(sl - 1, pa[0], pa[1])
                stA[sl] = a
            pa = stA.pop(SLABS - 1)
            stageB(SLABS - 1, pa[0], pa[1])

            octx.__exit__(None, None, None)
            wctx.__exit__(None, None, None)
            actx.__exit__(None, None, None)
            dctx.__exit__(None, None, None)

    nc.compile()
    return nc


def kernel(src, theta):
    if "prog" not in _CACHE:
        _CACHE["prog"] = _build_program()
    nc = _CACHE["prog"]
    in_maps = []
    for i in range(N):
        in_maps.append({
            "src": np.ascontiguousarray(src[i].reshape(C, SRCEL),
                                        dtype=np.float32),
            "theta": np.ascontiguousarray(theta[i].reshape(1, 12),
                                          dtype=np.float32),
        })
    res = run_bass_kernel_spmd(nc, in_maps, core_ids=list(range(N)))
    o = np.empty((N, C, D, H, W), dtype=np.float32)
    for i in range(N):
        o[i] = res.results[i]["out"].reshape(C, D, H, W)
    return o


# revision 13
# speedup vs baseline: 2.8238x; 1.1665x over previous
"""Trainium2 Bass kernel: 3D affine spatial transformer (affine_grid +
trilinear grid_sample, align_corners=True, zeros padding).

Data parallel: one sample per NeuronCore (8 cores). Per core, output is
processed per z-slab (96 slabs); a slab's 25600 output pixels form 128
lanes x 20 tasks of 10 consecutive x-pixels. A z- and y-duplicated,
zero-padded fp16 copy of src in DRAM (PV[zq, y, x][jz, c, jy]) lets ONE
indirect-DMA descriptor per task fetch the full (3z x 4y x 13x x 2c)
interpolation window as a 312-element contiguous stream (vs one
descriptor per (task, jz) in the previous version -> 6x fewer SWDGE
instructions, which were the bottleneck at ~1us each on the GpSimd Q7).
Blend is factored x-first: XB = sum_s hx(i,s)*D[i+s,:], then
out = sum_{jz,jy} hz*hy*XB -- exact trilinear with per-pixel hats.
"""

import numpy as np

import concourse.bass as bass
import concourse.bacc as bacc
import concourse.mybir as mybir
from concourse import tile
from concourse.bass import AP
from concourse.bass_utils import run_bass_kernel_spmd

F32 = mybir.dt.float32
F16 = mybir.dt.float16
I32 = mybir.dt.int32
AO = mybir.AluOpType
AF = mybir.ActivationFunctionType

N, C, D, H, W = 8, 2, 96, 160, 160
HW = H * W
SRCEL = D * H * W
OUTEL = C * SRCEL

LX, KX, KY, KZ = 10, 5, 4, 3
U = LX - 1 + KX                      # 14
INNER = KZ * C * KY                  # 24 per (zq,y,x) row: [jz][c][jy]
DSTREAM = U * INNER                  # 336 els per task descriptor
PADZ, PADY, PADX = 4, 6, 11
ZPN, YPN, XN = 100, 168, 185
JZSTR = YPN * XN                     # 29700 rows per z plane
ROWS = ZPN * JZSTR                   # 2940300
ZLIM, YLIM, XLIM = 99.0, 167.0, 171.0

NT = 20                              # tasks per lane per slab
TPR = W // LX                        # 16 tasks per output row
SLABS = D
CH_SL = 12                           # slabs per residual-phase chunk
NCHUNK = SLABS // CH_SL
TPC = CH_SL * NT                     # 240 tasks/lane/chunk
NTL = SLABS * NT                     # 1920 tasks per lane

HZB, HYB, HXB = 0, KZ * LX, KZ * LX + KY * LX   # 0, 30, 70
PSTR = INNER * KX                                # 120
TSTR4 = LX * PSTR                                # 1200
HATN = (KZ + KY + KX) * LX                       # 120

_CACHE = {}


def fb(apx, pairs, extra_off=0):
    """clone AP keeping partition pair, replacing free pairs"""
    return AP(apx.tensor, apx.offset + extra_off,
              [list(apx.ap[0])] + [list(p) for p in pairs])


def _build_program():
    P = 128
    nc = bacc.Bacc(None, target_bir_lowering=False)
    src = nc.declare_dram_parameter("src", [C, SRCEL], F32, isOutput=False)
    theta = nc.declare_dram_parameter("theta", [1, 12], F32, isOutput=False)
    out = nc.declare_dram_parameter("out", [1, OUTEL], F32, isOutput=True)
    src16p = nc.dram_tensor("src16p", [C, (D + PADZ + 2) * HW], F16)
    pv = nc.dram_tensor("pv", [ROWS, INNER], F16)

    r = 79.5 / 47.5

    with tile.TileContext(nc) as tc:
        with (
            tc.tile_pool(name="per", bufs=1) as per,
            tc.tile_pool(name="pp", bufs=1, space="PSUM") as pp,
        ):
            pre_ctx = tc.tile_pool(name="pre", bufs=2)
            pre = pre_ctx.__enter__()

            # ---------- P2: scalars, templates ----------
            th0 = per.tile([P, 12], F32)
            nc.sync.dma_start(out=th0[:1, :], in_=theta[:])
            ones1 = per.tile([1, P], F32)
            nc.vector.memset(ones1[:], 1.0)
            thps = pp.tile([P, 12], F32, tag="thps")
            nc.tensor.matmul(out=thps[:], lhsT=ones1[:], rhs=th0[:1, :],
                             start=True, stop=True)
            thb = per.tile([P, 12], F32)
            nc.vector.tensor_copy(out=thb[:], in_=thps[:])

            def thc(j):
                return thb[:, j:j + 1]

            # per-axis scalars A,B,C,O' (O' includes +pad)
            sc = per.tile([P, 24], F32)
            (AZ, BZ, CZ, OZ, AY, BY, CY, OY, AX, BX, CX, OX, AXM1,
             MZB, MYB, MXB) = range(16)

            def scc(j):
                return sc[:, j:j + 1]

            tmp1 = per.tile([P, 1], F32, tag="tmp1")

            def mkrow(dst, srci, cmul, osc, oadd):
                nc.vector.tensor_copy(out=scc(dst[0]), in_=thc(srci))
                nc.vector.tensor_copy(out=scc(dst[1]), in_=thc(srci + 1))
                nc.vector.tensor_scalar_mul(out=scc(dst[2]),
                                            in0=thc(srci + 2), scalar1=cmul)
                nc.vector.tensor_tensor(out=tmp1[:], in0=thc(srci + 3),
                                        in1=thc(srci), op=AO.subtract)
                nc.vector.tensor_tensor(out=tmp1[:], in0=tmp1[:],
                                        in1=thc(srci + 1), op=AO.subtract)
                nc.vector.tensor_tensor(out=tmp1[:], in0=tmp1[:],
                                        in1=thc(srci + 2), op=AO.subtract)
                nc.vector.tensor_scalar(out=scc(dst[3]), in0=tmp1[:],
                                        scalar1=osc, scalar2=osc + oadd,
                                        op0=AO.mult, op1=AO.add)

            mkrow((AX, BX, CX, OX), 0, r, 79.5, float(PADX))
            mkrow((AY, BY, CY, OY), 4, r, 79.5, float(PADY))
            mkrow((AZ, BZ, CZ, OZ), 8, 1.0, 47.5, float(PADZ))
            # z row: A=t20/r, B=t21/r, C=t22
            nc.vector.tensor_scalar_mul(out=scc(AZ), in0=thc(8),
                                        scalar1=1.0 / r)
            nc.vector.tensor_scalar_mul(out=scc(BZ), in0=thc(9),
                                        scalar1=1.0 / r)
            nc.vector.tensor_scalar_add(out=scc(AXM1), in0=scc(AX),
                                        scalar1=-1.0)

            def mkminb(dst, slope_i):
                nc.vector.tensor_scalar_mul(out=scc(dst), in0=scc(slope_i),
                                            scalar1=float(LX - 1))
                nc.vector.tensor_scalar_min(out=scc(dst), in0=scc(dst),
                                            scalar1=0.0)

            mkminb(MZB, AZ)
            mkminb(MYB, AY)
            mkminb(MXB, AXM1)

            # lane mapping: flat x-chunk index g = NT*p + t;
            # y = g // TPR, xc = g % TPR, x0 = LX*xc
            def iotaf(shape_pairs, n, tag, base=0, cm=0):
                ti_ = pre.tile([P, n], I32, tag=tag + "i")
                nc.gpsimd.iota(ti_[:], shape_pairs, base=base,
                               channel_multiplier=cm)
                tf_ = per.tile([P, n], F32, tag=tag + "f")
                nc.vector.tensor_copy(out=tf_[:], in_=ti_[:])
                return tf_

            gT = iotaf([[1, NT]], NT, "g", cm=NT)
            yT = per.tile([P, NT], F32)
            gi1 = pre.tile([P, NT], I32, tag="gi1")
            yv = pre.tile([P, NT], F32, tag="yv")
            nc.vector.tensor_scalar_mul(out=yv[:], in0=gT[:],
                                        scalar1=1.0 / TPR)
            nc.vector.tensor_copy(out=gi1[:], in_=yv[:])
            nc.vector.tensor_copy(out=yT[:], in_=gi1[:])
            ygt = pre.tile([P, NT], F32, tag="ygt")
            nc.vector.tensor_tensor(out=ygt[:], in0=yT[:], in1=yv[:],
                                    op=AO.is_gt)
            nc.vector.tensor_tensor(out=yT[:], in0=yT[:], in1=ygt[:],
                                    op=AO.subtract)
            x0T = per.tile([P, NT], F32)
            nc.vector.scalar_tensor_tensor(out=x0T[:], in0=yT[:],
                                           scalar=-float(TPR), in1=gT[:],
                                           op0=AO.mult, op1=AO.add)
            nc.vector.tensor_scalar_mul(out=x0T[:], in0=x0T[:],
                                        scalar1=float(LX))

            # base[axis] = A*x0 + B*y + O'   [P, NT]
            base3 = per.tile([P, 3, NT], F32)
            for ax, (ai, bi, oi) in enumerate(((AZ, BZ, OZ), (AY, BY, OY),
                                               (AX, BX, OX))):
                b1 = pre.tile([P, NT], F32, tag="b1")
                nc.vector.scalar_tensor_tensor(
                    out=b1[:], in0=x0T[:], scalar=scc(ai),
                    in1=fb(sc[:], [[0, NT]], oi), op0=AO.mult, op1=AO.add)
                nc.vector.scalar_tensor_tensor(
                    out=base3[:, ax, :], in0=yT[:], scalar=scc(bi),
                    in1=b1[:], op0=AO.mult, op1=AO.add)

            # hat templates T[ax][k][i] = slope_ax*i - k, fp16 [P, HATN]
            Tf = per.tile([P, HATN], F32)
            for axoff, kcnt, si in ((HZB, KZ, AZ), (HYB, KY, AY),
                                    (HXB, KX, AXM1)):
                nb_ = kcnt * LX
                iw = iotaf([[0, kcnt], [1, LX]], nb_, f"iw{axoff}")
                ik = iotaf([[1, kcnt], [0, LX]], nb_, f"ik{axoff}")
                nc.vector.scalar_tensor_tensor(
                    out=Tf[:, axoff:axoff + nb_], in0=iw[:], scalar=scc(si),
                    in1=ik[:], op0=AO.mult, op1=AO.subtract)
            T110 = per.tile([P, HATN], F16)
            nc.vector.tensor_copy(out=T110[:], in_=Tf[:])

            # ---------- P3: per-task residuals (fp16) + PV row index ----
            rT = per.tile([P, NTL, 3], F16)
            idxT = per.tile([P, NTL], F32)
            lims = (ZLIM, YLIM, XLIM)
            mbs = (MZB, MYB, MXB)
            cxs = (CZ, CY, CX)
            for ch in range(NCHUNK):
                zoi = pre.tile([P, TPC], I32, tag="zoi")
                nc.gpsimd.iota(zoi[:], [[1, CH_SL], [0, NT]],
                               base=ch * CH_SL, channel_multiplier=0)
                zof = pre.tile([P, TPC], F32, tag="zof")
                nc.vector.tensor_copy(out=zof[:], in_=zoi[:])
                acc = idxT[:, ch * TPC:(ch + 1) * TPC]
                for ax in range(3):
                    cs = pre.tile([P, TPC], F32, tag="cs")
                    nc.vector.scalar_tensor_tensor(
                        out=cs[:], in0=zof[:], scalar=scc(cxs[ax]),
                        in1=fb(base3[:, ax, :], [[0, CH_SL], [1, NT]]),
                        op0=AO.mult, op1=AO.add)
                    bf = pre.tile([P, TPC], F32, tag="bf")
                    bv = pre.tile([P, TPC], F32, tag="bv")
                    bi_ = pre.tile([P, TPC], I32, tag="bi")
                    nc.vector.tensor_scalar_add(out=bv[:], in0=cs[:],
                                                scalar1=scc(mbs[ax]))
                    nc.vector.tensor_copy(out=bi_[:], in_=bv[:])
                    nc.vector.tensor_copy(out=bf[:], in_=bi_[:])
                    bg = pre.tile([P, TPC], F32, tag="bg")
                    nc.vector.tensor_tensor(out=bg[:], in0=bf[:], in1=bv[:],
                                            op=AO.is_gt)
                    nc.vector.tensor_tensor(out=bf[:], in0=bf[:], in1=bg[:],
                                            op=AO.subtract)
                    nc.vector.tensor_scalar_max(out=bf[:], in0=bf[:],
                                                scalar1=0.0)
                    nc.vector.tensor_scalar_min(out=bf[:], in0=bf[:],
                                                scalar1=lims[ax])
                    nc.vector.tensor_tensor(
                        out=fb(rT[:], [[3, TPC]], (ch * TPC) * 3 + ax),
                        in0=cs[:], in1=bf[:], op=AO.subtract)
                    if ax == 0:
                        nc.vector.tensor_scalar_mul(out=acc, in0=bf[:],
                                                    scalar1=float(JZSTR))
                    elif ax == 1:
                        nc.vector.scalar_tensor_tensor(
                            out=acc, in0=bf[:], scalar=float(XN), in1=acc,
                            op0=AO.mult, op1=AO.add)
                    else:
                        nc.vector.tensor_tensor(out=acc, in0=acc, in1=bf[:],
                                                op=AO.add)

            # ---------- P1: cast src to fp16 (z-padded), build PV ----------
            # src16p[c] = 3 zero planes | 96 data planes | 2 zero planes
            DP5 = D + PADZ + 2
            zt = pre.tile([P, 800], F16)
            nc.vector.memset(zt[:], 0.0)
            for c in range(C):
                nc.sync.dma_start(
                    out=AP(src16p[:].tensor, c * DP5 * HW,
                           [[1, PADZ * HW]]),
                    in_=fb(zt[:], [[1, PADZ * HW // P]]))
                nc.sync.dma_start(
                    out=AP(src16p[:].tensor, c * DP5 * HW + (PADZ + D) * HW,
                           [[1, 2 * HW]]),
                    in_=fb(zt[:], [[1, 2 * HW // P]]))
            nc.gpsimd.dma_start(
                out=AP(src16p[:].tensor, PADZ * HW,
                       [[DP5 * HW, C], [1, SRCEL]]),
                in_=src[:])

            # PV row (zq, y, x)[jz][c][jy] =
            #   src16p[c, zq+jz, y-PADY+jy, x-PADX]   (padded planes)
            YB = 6
            nbands = (YPN + YB - 1) // YB
            for bidx in range(nbands):
                B = bidx * YB
                nb = min(YB, YPN - B)
                scs = []
                r0 = B - PADY                # src row at jy=0
                rlo = max(0, r0)
                rhi = min(H, r0 + nb + KY - 1)
                for c in range(C):
                    for jz in range(KZ):
                        # partition p of sct holds padded plane p + jz
                        sct = pre.tile([ZPN, (YB + KY - 1) * W], F16,
                                       tag=f"sc{c}_{jz}")
                        if rlo > r0 or rhi < r0 + nb + KY - 1:
                            nc.vector.memset(sct[:], 0.0)
                        if rhi > rlo:
                            nc.sync.dma_start(
                                out=sct[:, (rlo - r0) * W:(rhi - r0) * W],
                                in_=AP(src16p[:].tensor,
                                       c * DP5 * HW + jz * HW + rlo * W,
                                       [[HW, ZPN],
                                        [1, (rhi - rlo) * W]]))
                        scs.append(sct)
                il = pre.tile([ZPN, YB * XN * INNER], F16, tag="il")
                # zero x-pad columns (x<PADX and x>=PADX+W)
                nc.vector.memset(
                    fb(il[:], [[XN * INNER, nb], [1, PADX * INNER]]), 0.0)
                nc.vector.memset(
                    fb(il[:], [[XN * INNER, nb],
                               [1, (XN - PADX - W) * INNER]],
                       (PADX + W) * INNER), 0.0)
                # interleave: 6 copies (c, jz), jy merged via stride-W pair
                for c in range(C):
                    for jz in range(KZ):
                        dst = fb(il[:], [[XN * INNER, nb], [INNER, W],
                                         [C, KY]],
                                 PADX * INNER + jz * C * KY + c)
                        srcap = fb(scs[c * KZ + jz][:],
                                   [[W, nb], [1, W], [W, KY]])
                        if (c * KZ + jz) % 2 == 0:
                            nc.scalar.activation(dst, srcap, AF.Copy)
                        else:
                            nc.vector.tensor_copy(out=dst, in_=srcap)
                nc.sync.dma_start(
                    out=AP(pv[:].tensor, B * XN * INNER,
                           [[JZSTR * INNER, ZPN], [1, nb * XN * INNER]]),
                    in_=fb(il[:], [[1, nb * XN * INNER]]))

            # ---------- P4: main loop over z-slabs ----------
            pre_ctx.__exit__(None, None, None)
            dctx = tc.tile_pool(name="dp", bufs=3)
            dp = dctx.__enter__()
            actx = tc.tile_pool(name="ap", bufs=2)
            apl = actx.__enter__()
            wctx = tc.tile_pool(name="wp", bufs=1)
            wp = wctx.__enter__()
            octx = tc.tile_pool(name="op", bufs=2)
            op = octx.__enter__()

            stA = {}

            def stageA(sl):
                offs = dp.tile([P, NT], I32, tag="offs")
                nc.vector.tensor_copy(
                    out=offs[:], in_=fb(idxT[:], [[1, NT]], sl * NT))
                Dt = dp.tile([P, NT * DSTREAM], F16, tag="Dt")
                for t in range(NT):
                    nc.gpsimd.indirect_dma_start(
                        out=Dt[:, t * DSTREAM:(t + 1) * DSTREAM],
                        out_offset=None, in_=pv[:],
                        in_offset=bass.IndirectOffsetOnAxis(
                            ap=offs[:, t:t + 1], axis=0))
                # args[t, ax-block] = T110 + r_ax ; hats = relu(1-|args|)
                args = apl.tile([P, NT, HATN], F16, tag="args")
                for axoff, kcnt, ax in ((HZB, KZ, 0), (HYB, KY, 1),
                                        (HXB, KX, 2)):
                    nb_ = kcnt * LX
                    nc.vector.tensor_tensor(
                        out=fb(args[:], [[HATN, NT], [1, nb_]], axoff),
                        in0=fb(T110[:], [[0, NT], [1, nb_]], axoff),
                        in1=fb(rT[:], [[3, NT], [0, nb_]], sl * NT * 3 + ax),
                        op=AO.add)
                habs = apl.tile([P, NT * HATN], F16, tag="habs")
                nc.scalar.activation(habs[:],
                                     args[:].rearrange("p a b -> p (a b)"),
                                     AF.Abs)
                hatt = apl.tile([P, NT * HATN], F16, tag="hatt")
                nc.scalar.activation(hatt[:], habs[:], AF.Relu,
                                     bias=1.0, scale=-1.0)
                return Dt, hatt

            def stageB(sl, Dt, hatt):
                # w2cd[t][i][jz][c][jy] = hz * hy   (c-duplicated)
                w2cd = wp.tile([P, NT * 240], F16, tag="w2cd")
                for jz in range(KZ):
                    for c in range(C):
                        nc.vector.tensor_tensor(
                            out=fb(w2cd[:], [[240, NT], [24, LX], [C, KY]],
                                   jz * C * KY + c),
                            in0=fb(hatt[:], [[HATN, NT], [1, LX], [0, KY]],
                                   HZB + jz * LX),
                            in1=fb(hatt[:], [[HATN, NT], [1, LX], [LX, KY]],
                                   HYB),
                            op=AO.mult)
                # G[t][s][i][(jz c jy)24] = D[i+s, :] * (hz*hy)   (all
                # stride-1 operands keep DVE in 2x mode)
                G = wp.tile([P, NT * TSTR4], F16, tag="G")
                for s in range(KX):
                    nc.vector.tensor_tensor(
                        out=fb(G[:], [[TSTR4, NT], [INNER, LX],
                                      [1, INNER]], s * LX * INNER),
                        in0=fb(Dt[:], [[DSTREAM, NT], [INNER, LX],
                                       [1, INNER]], s * INNER),
                        in1=fb(w2cd[:], [[240, NT], [24, LX], [1, INNER]]),
                        op=AO.mult)
                with nc.allow_low_precision(reason="fp16 trilinear accum"):
                    # R1[t][(s i jz c)300] = sum_jy G  (pairwise adds,
                    # c-innermost keeps step-1 operands in 2x mode)
                    Rh = wp.tile([P, NT * KX * 120], F16, tag="Rh")
                    nc.vector.tensor_tensor(
                        out=Rh[:],
                        in0=fb(G[:], [[TSTR4, NT], [4, 300], [1, 2]]),
                        in1=fb(G[:], [[TSTR4, NT], [4, 300], [1, 2]], 2),
                        op=AO.add)
                    R1 = wp.tile([P, NT * KX * 60], F16, tag="R1")
                    nc.vector.tensor_tensor(
                        out=R1[:],
                        in0=fb(Rh[:], [[KX * 120, NT], [4, 150], [1, 2]]),
                        in1=fb(Rh[:], [[KX * 120, NT], [4, 150], [1, 2]], 2),
                        op=AO.add)
                    # X3[t][(i jz c)60] = sum_s hx(i,s) * R1[s]
                    Hp = wp.tile([P, NT * KX * 60], F16, tag="Hp")
                    for s in range(KX):
                        nc.vector.tensor_tensor(
                            out=fb(Hp[:], [[KX * 60, NT], [6, LX], [1, 6]],
                                   s * 60),
                            in0=fb(R1[:], [[KX * 60, NT], [6, LX], [1, 6]],
                                   s * 60),
                            in1=fb(hatt[:], [[HATN, NT], [1, LX], [0, 6]],
                                   HXB + s * LX),
                            op=AO.mult)
                    X3 = wp.tile([P, NT * 60], F16, tag="X3")
                    nc.vector.tensor_tensor(
                        out=X3[:],
                        in0=fb(Hp[:], [[KX * 60, NT], [1, 60]]),
                        in1=fb(Hp[:], [[KX * 60, NT], [1, 60]], 60),
                        op=AO.add)
                    for s in range(2, KX):
                        nc.vector.tensor_tensor(
                            out=X3[:], in0=X3[:],
                            in1=fb(Hp[:], [[KX * 60, NT], [1, 60]], s * 60),
                            op=AO.add)
                    # otile[c][t][i] = sum_jz X3   (f32)
                    otile = op.tile([P, C * NT * LX], F32, tag="otile")
                    for c in range(C):
                        nc.vector.tensor_reduce(
                            out=fb(otile[:], [[LX, NT], [1, LX]],
                                   c * NT * LX),
                            in_=fb(X3[:], [[60, NT], [6, LX], [2, KZ]], c),
                            op=AO.add, axis=mybir.AxisListType.X)
                for c in range(C):
                    nc.sync.dma_start(
                        out=AP(out[:].tensor, c * SRCEL + sl * HW,
                               [[NT * LX, P], [1, NT * LX]]),
                        in_=fb(otile[:], [[1, NT * LX]], c * NT * LX))

            for sl in range(SLABS):
                a = stageA(sl)
                if sl > 0:
                    pa = stA.pop(sl - 1)
                    stageB(sl - 1, pa[0], pa[1])
                stA[sl] = a
            pa = stA.pop(SLABS - 1)
            stageB(SLABS - 1, pa[0], pa[1])

            octx.__exit__(None, None, None)
            wctx.__exit__(None, None, None)
            actx.__exit__(None, None, None)
            dctx.__exit__(None, None, None)

    nc.compile()
    return nc


def kernel(src, theta):
    if "prog" not in _CACHE:
        _CACHE["prog"] = _build_program()
    nc = _CACHE["prog"]
    in_maps = []
    for i in range(N):
        in_maps.append({
            "src": np.ascontiguousarray(src[i].reshape(C, SRCEL),
                                        dtype=np.float32),
            "theta": np.ascontiguousarray(theta[i].reshape(1, 12),
                                          dtype=np.float32),
        })
    res = run_bass_kernel_spmd(nc, in_maps, core_ids=list(range(N)))
    o = np.empty((N, C, D, H, W), dtype=np.float32)
    for i in range(N):
        o[i] = res.results[i]["out"].reshape(C, D, H, W)
    return o
